# revision 43
# baseline (speedup 1.0000x reference)
"""Trainium2 Bass kernel for nn_FullAttention_17789754540074.

Self-contained: takes the FULL inputs of reference.setup_inputs(), returns the
FULL output. Internally shards across 8 NeuronCores as 2-way data parallel
(batch) x 4-way tensor parallel (3 heads + 384 FF pairs per rank), runs one
SPMD Bass/Tile program via run_bass_kernel_spmd, and sums the 4 partial
outputs per batch on the host, adding the residual there too (the unshard
step for partial-sum TP sharding).

v2 structure (vs the 161us baseline):
  - x ships fp8; the RMS-norm scale is folded into the fused/v WEIGHTS
    (1152+208 cols per chunk on the scale pass instead of 1728), so the
    QKV/ff/v matmuls consume raw fp8 x directly
  - inputs arrive in 15 DMAs (constant packs by dtype + priority) instead of
    47; outputs leave in 12 per-o row DMAs instead of 24 quarter DMAs
  - softmax: all score pairs land in [128,2,512] PSUM tiles; ACT pairs use a
    single merged exp per pair; DVE pairs use a uint8 Schraudolph writing
    fp8e4 probs directly (saturates at 0 below, bits<=126 above by XBIAS
    choice), so ALL A*V matmuls run fp8 DoubleRow and the bf16 v copy dies
  - per-slot normalize fused: dact = av[0:64] * pob with both operands in
    PSUM (no separate att evacuation)
  - rope cos/sin muls run bf16 2x on DVE instead of Pool
"""

import math

import numpy as np

import concourse.bass as bass
import concourse.mybir as mybir
import concourse.tile as tile
from concourse import bass_utils
from concourse.vector_clock import ScopedClock

F32 = mybir.dt.float32
F32R = mybir.dt.float32r
F8 = mybir.dt.float8e4
BF16 = mybir.dt.bfloat16
U8 = mybir.dt.uint8
AF = mybir.ActivationFunctionType
ALU = mybir.AluOpType
DR = mybir.MatmulPerfMode.DoubleRow

HID, HEADS, HD, MLP = 768, 12, 64, 3072
B, H, W, D = 2, 12, 12, 12
S = H * W * D  # 1728
ROT = 48
MAX_FREQ = 256.0
EPS_GN, EPS_LN = 1e-6, 1e-5

N_CORES = 8
TP = 4
HPC = 3  # heads per core
FFPC = 384  # ff pairs per core
NF = 9 * 128  # fused rows: q01 | k01 | q2k2 | ffx*3 | gate*3
VP = 208  # v proj cols: [v0|1|v1|1|v2|1] = 195 used + pad

S_BLOCKS = [(0, 512), (512, 512), (1024, 448), (1472, 256)]
T_TILES = [(128 * j, 128) for j in range(13)] + [(1664, 64)]
QF = S // 4  # 432 queries per attention slot
FB_BLOCKS = [(QF * q, QF) for q in range(4)]
TR_BLOCKS = [(0, 512), (512, 512), (1024, 512), (1536, 192)]

# Softmax exp bias: keeps Schraudolph uint8 bits <= ~120 (below the 0x7F NaN
# encoding) for scores up to ~8, while bits<0 saturate to 0 (prob 0), roughly
# matching the fp8 subnormal flush of the ACT exp path.
XBIAS = -2.5
# fp8e4m3 Schraudolph: u8 = sc*ACOEF8 + BCONST8; bits(u8) ~ e4m3(exp(sc+XBIAS))
ACOEF8 = float(8.0 / math.log(2.0))
BCONST8 = float(8.0 * (XBIAS / math.log(2.0) + 7.0) - 0.490)
# which key-tile pairs per slot run on DVE (rest on ACT)
DVE_PAIRS = (1, 3, 5)
# pair emission order within a slot: alternate ACT/DVE consumers, start with
# the odd p=6 pair so the slot tail ends on fast pairs
PAIR_ORDER = (6, 1, 0, 3, 2, 5, 4)
# emit interleaved work at these POSITIONS in PAIR_ORDER (the DVE pairs, so
# the works' ACT/PE load lands while ACT is otherwise idle)
WORK_AT = (1, 3, 5)


# constant-pack column layout (see _prep_core_inputs)
PS_NW = (0, 6)
PS_SELQ = (6, 134)
PS_SELK = (134, 262)
PS_SEL2 = (262, 390)
PS_E164 = (390, 454)
PS_EPS4 = (454, 455)  # rows: EPS, EPS, 64*EPS, 64*EPS
PS_EPS2 = (455, 456)  # rows: EPS, 64*EPS
PS_COLS = 456
PB_WQ4 = (0, 4)  # cols (iq_q0, iq_q1, 0, 0)
PB_WK4 = (4, 8)  # cols (0, 0, ik_k0, ik_k1)
PB_WQK2 = (8, 10)
PB_COLS = 10
PR_RR = (0, 128)
PR_COS = (128, 128 + S)
PR_SIN = (128 + S, 128 + 2 * S)
PR_COLS = 128 + 2 * S


class TileContextSplitDrain(tile.TileContext):
    """TileContext whose kernel-tail drain splits its semaphore waits across
    single-wait sync NOPs — the walrus build here rejects >2 sync waits on one
    SP CTRL instruction ("Too many sync wait commands")."""

    def _drain_and_barrier(self, tick_clock, wait_clock):
        probe = self.nc.sync.nop(nofuse=True)
        wait_clock.add_sem_waits(
            probe.ins, ScopedClock({None: tick_clock.global_clock})
        )
        si = probe.ins.sync_info
        waits = list(si.on_wait) if si is not None else []
        if si is not None:
            si.on_wait = waits[:1]
        for w in waits[1:]:
            n = self.nc.sync.nop(nofuse=True)
            nsi = n.ins.sync_info
            if nsi is None:
                n.ins.sync_info = mybir.SyncInfo(on_wait=[w], on_update=[])
            else:
                nsi.on_wait.append(w)
        self.nc.sync.drain()
        self.nc.all_engine_barrier()
        popped = self.nc._tile_sem_poison_stack.pop()
        assert popped is self._sem_poison
        self.nc.clear_and_free_semaphores(list(self.sems.allocated().values()))
        self.nc.all_engine_barrier()


def r32(ap):
    return ap.bitcast(F32R)


def _split_excess_waits(nc, maxw=1):
    """walrus in this container caps sync waits per instruction; move extras
    onto preceding same-engine NOPs (waits execute in program order)."""
    nid = 0
    for bb in nc.m.functions[0].blocks:
        insts = bb.instructions
        i = 0
        while i < len(insts):
            inst = insts[i]
            si = inst.sync_info
            nw = len(si.on_wait) if si is not None and si.on_wait else 0
            if nw > maxw:
                waits = list(si.on_wait)
                si.on_wait = waits[-maxw:]
                extra = waits[:-maxw]
                pos = i
                for k in range(0, len(extra), maxw):
                    nop = mybir.InstNoOp(
                        name=f"I-waitsplit-{nid}", ins=[], outs=[]
                    )
                    nop.engine = inst.engine
                    nop.sync_info = mybir.SyncInfo(
                        on_wait=extra[k : k + maxw], on_update=[]
                    )
                    insts.insert(pos, nop)
                    nc.register_instruction(nop)
                    pos += 1
                    i += 1
                    nid += 1
            i += 1


def build_program():
    nc = bass.Bass(trn_type="TRN2")

    xT = nc.dram_tensor("xT", [128, 6, S], F8, kind="ExternalInput")
    wfT = nc.dram_tensor("wfT", [128, 6, NF], BF16, kind="ExternalInput")
    wvT = nc.dram_tensor("wvT", [128, 6, VP], BF16, kind="ExternalInput")
    wa4T = nc.dram_tensor("wa4T", [64, 4, HID], F8, kind="ExternalInput")
    wfbT = nc.dram_tensor("wfbT", [128, 4, HID], F8, kind="ExternalInput")
    packST = nc.dram_tensor("packST", [128, PS_COLS], F32, kind="ExternalInput")
    packBT = nc.dram_tensor("packBT", [128, PB_COLS], BF16, kind="ExternalInput")
    packRT = nc.dram_tensor("packRT", [128, PR_COLS], BF16, kind="ExternalInput")
    outT = nc.dram_tensor("outT", [HID, S], BF16, kind="ExternalOutput")

    with TileContextSplitDrain(nc) as tc:
        with tc.tile_pool(name="main", bufs=1) as pm:
            # ---- long-lived SBUF tiles --------------------------------------
            xraw = pm.tile([128, 6, S], F8, name="xraw", tag="xraw")
            wf16 = pm.tile([128, 6, NF], BF16, name="wf16", tag="wf16")
            wv16 = pm.tile([128, 6, VP], BF16, name="wv16", tag="wv16")
            wf8s = pm.tile([128, 6, NF], F8, name="wf8s", tag="wf8s")
            wv8s = pm.tile([128, 6, VP], F8, name="wv8s", tag="wv8s")
            wa4 = pm.tile([64, 4, HID], F8, name="wa4", tag="wa4")
            wfb = pm.tile([128, 4, HID], F8, name="wfb", tag="wfb")
            packS = pm.tile([128, PS_COLS], F32, name="packS", tag="packS")
            packB = pm.tile([128, PB_COLS], BF16, name="packB", tag="packB")
            packR = pm.tile([128, PR_COLS], BF16, name="packR", tag="packR")
            qab = pm.tile([128, S], BF16, name="qab", tag="qab")
            kab = pm.tile([128, S], BF16, name="kab", tag="kab")
            qk2 = pm.tile([128, S], BF16, name="qk2", tag="qk2")
            qst = pm.tile([128, S], F8, name="qst", tag="qst")
            kst = pm.tile([128, S], F8, name="kst", tag="kst")
            q2st = pm.tile([128, S], F8, name="q2st", tag="q2st")
            k2q = pm.tile([64, S], F8, name="k2q", tag="k2q")
            vx8 = pm.tile([128, 14, VP], F8, name="vx8", tag="vx8")
            dact = pm.tile([HD, 4, S], F8, name="dact", tag="dact")
            dff = pm.tile([128, 4, S], F8, name="dff", tag="dff")
            sqq = pm.tile([128, S], BF16, name="sqq", tag="sqq")
            sqk = pm.tile([128, S], BF16, name="sqk", tag="sqk")
            sq2 = pm.tile([128, S], BF16, name="sq2", tag="sq2")
            sqju = pm.tile([128, 2, S], F8, name="sqju", tag="sqju")
            lnr = pm.tile([4, S], F32, name="lnr", tag="lnr")
            lnr2 = pm.tile([2, S], F32, name="lnr2", tag="lnr2")
            rrow = pm.tile([4, S], F32, name="rrow", tag="rrow")
            rrow2 = pm.tile([2, S], F32, name="rrow2", tag="rrow2")
            ss12 = pm.tile([128, 6], F32, name="ss12", tag="ss12")
            rmsc = pm.tile([128, 6], F32, name="rmsc", tag="rmsc")
            scl6 = pm.tile([128, 6], F32, name="scl6", tag="scl6")
            cgn = pm.tile([128, 1], F32, name="cgn", tag="cgn")
            cm8 = pm.tile([128, 1], F32, name="cm8", tag="cm8")
            tsq = pm.tile([128, S], BF16, name="tsq", tag="tsq")
            tcq = pm.tile([128, S], BF16, name="tcq", tag="tcq")
            tsk = pm.tile([128, S], BF16, name="tsk", tag="tsk")
            tck = pm.tile([128, S], BF16, name="tck", tag="tck")
            obr = pm.tile([128, 6, S], BF16, name="obr", tag="obr")

            # constant-pack slices
            nw = packS[:, PS_NW[0] : PS_NW[1]]
            sel_q = r32(packS[0:4, PS_SELQ[0] : PS_SELQ[1]])
            sel_k = r32(packS[0:4, PS_SELK[0] : PS_SELK[1]])
            sel_2 = r32(packS[0:2, PS_SEL2[0] : PS_SEL2[1]])
            e164 = r32(packS[0:1, PS_E164[0] : PS_E164[1]])
            wq4 = packB[:, PB_WQ4[0] : PB_WQ4[1]]
            wk4 = packB[:, PB_WK4[0] : PB_WK4[1]]
            wqk2 = packB[:, PB_WQK2[0] : PB_WQK2[1]]
            eps4 = packS[0:4, PS_EPS4[0] : PS_EPS4[1]]
            eps2 = packS[0:2, PS_EPS2[0] : PS_EPS2[1]]
            rrm = packR[:, PR_RR[0] : PR_RR[1]]
            cosb = packR[:, PR_COS[0] : PR_COS[1]]
            sinb = packR[:, PR_SIN[0] : PR_SIN[1]]

            # ---- input DMAs (priority order) --------------------------------
            nc.sync.dma_start(packS[:], packST[:])
            for c in range(6):
                nc.sync.dma_start(xraw[:, c, :], xT[:, c, :])
            nc.sync.dma_start(packB[:], packBT[:])
            for p in range(3):
                nc.sync.dma_start(
                    wf16[:, 2 * p : 2 * p + 2, :], wfT[:, 2 * p : 2 * p + 2, :]
                )
            nc.sync.dma_start(packR[:], packRT[:])
            nc.sync.dma_start(wv16[:], wvT[:])
            nc.sync.dma_start(wa4[:], wa4T[:])
            nc.sync.dma_start(wfb[:], wfbT[:])

            # ---- constants / zero pads --------------------------------------
            nc.vector.memset(cgn[:], EPS_GN)
            nc.vector.memset(cm8[:], XBIAS)

            nc.gpsimd.memset(dact[:, 3, :], 0.0)
            nc.gpsimd.memset(dff[:, 3, :], 0.0)


            # ---- phase A: rms norm stats + weight scaling + fused qkv -------
            with (
                tc.tile_pool(name="psA", bufs=3, space="PSUM") as psA,
                tc.tile_pool(name="psRP", bufs=3, space="PSUM") as psRP,
            ):
                for c in range(6):
                    nc.scalar.activation(
                        sqju[:, c % 2, :], xraw[:, c, :], AF.Square,
                        accum_out=ss12[:, c : c + 1],
                    )
                    nc.scalar.activation(
                        rmsc[:, c : c + 1], ss12[:, c : c + 1], AF.Sqrt,
                        bias=cgn[:], scale=1.0 / S,
                    )
                    nc.vector.reciprocal(
                        scl6[:, c : c + 1], rmsc[:, c : c + 1]
                    )
                    nc.vector.tensor_mul(
                        scl6[:, c : c + 1], scl6[:, c : c + 1], nw[:, c : c + 1]
                    )
                    # scale weights (not x): wf8s = fp8(wf16 * scl), ditto wv
                    weng = (nc.scalar, nc.vector, nc.scalar,
                            nc.vector, nc.scalar, nc.vector)[c]
                    if weng is nc.scalar:
                        nc.scalar.activation(
                            wf8s[:, c, :], wf16[:, c, :], AF.Copy,
                            scale=scl6[:, c : c + 1],
                        )
                    else:
                        weng.tensor_scalar(
                            wf8s[:, c, :], wf16[:, c, :],
                            scl6[:, c : c + 1], None, ALU.mult,
                        )
                    nc.gpsimd.tensor_scalar(
                        wv8s[:, c, :], wv16[:, c, :],
                        scl6[:, c : c + 1], None, ALU.mult,
                    )

                # qkv q01/k01 blocks first (o=0,1), evacs spread over engines
                qk_dst = [qab, kab, qk2]

                def qkv_block(o, sb):
                    soff, slen = S_BLOCKS[sb]
                    pt = psA.tile([128, 512], F32, name="mm", tag="mm")
                    acc = pt[:, :slen]
                    for p in range(3):
                        nc.tensor.matmul(
                            acc,
                            wf8s[:, 2 * p : 2 * p + 2,
                                 128 * o : 128 * (o + 1)],
                            xraw[:, 2 * p : 2 * p + 2, soff : soff + slen],
                            start=(p == 0), stop=(p == 2), perf_mode=DR,
                        )
                    eng = (nc.vector, nc.scalar)[(o + sb) % 2]
                    if eng is nc.scalar:
                        nc.scalar.activation(
                            qk_dst[o][:, soff : soff + slen], acc, AF.Copy
                        )
                    else:
                        eng.tensor_copy(qk_dst[o][:, soff : soff + slen], acc)

                for o in range(2):
                    for sb in range(4):
                        qkv_block(o, sb)

                # ---- rope sin/cos parts (no stats dependency) ---------------
                # per-block: sin-mul on DVE (reads PSUM), cos-mul on Pool,
                # add on DVE (bf16 2x) — q and k chains overlap; the squares
                # for the token stats ride per-block on ACT
                for src, ts_t, tc_t, sq_t in (
                    (qab, tsq, tcq, sqq), (kab, tsk, tck, sqk)
                ):
                    for soff, slen in S_BLOCKS:
                        rot = psRP.tile([128, 512], F32, name="rot", tag="rp")
                        nc.tensor.matmul(
                            rot[:, :slen], rrm, src[:, soff : soff + slen]
                        )
                        nc.gpsimd.tensor_mul(
                            tc_t[:, soff : soff + slen],
                            src[:, soff : soff + slen],
                            cosb[:, soff : soff + slen],
                        )
                        nc.scalar.activation(
                            sq_t[:, soff : soff + slen],
                            src[:, soff : soff + slen], AF.Square,
                        )
                        nc.vector.tensor_mul(
                            ts_t[:, soff : soff + slen],
                            rot[:, :slen],
                            sinb[:, soff : soff + slen],
                        )
                        nc.vector.tensor_add(
                            ts_t[:, soff : soff + slen],
                            ts_t[:, soff : soff + slen],
                            tc_t[:, soff : soff + slen],
                        )

                # ---- q01/k01 rstd stats, directly in row layout -------------
                # per-token variance sums land as [4, block] rows via
                # moving=squares matmuls; rstd = exp(-0.5 * ln(sum + eps))
                for soff, slen in S_BLOCKS:
                    pr = psA.tile([4, 512], F32, name="vk", tag="mm")
                    nc.tensor.matmul(
                        pr[:, :slen], wq4, sqq[:, soff : soff + slen],
                        start=True, stop=False,
                    )
                    nc.tensor.matmul(
                        pr[:, :slen], wk4, sqk[:, soff : soff + slen],
                        start=False, stop=True,
                    )
                    nc.scalar.activation(
                        lnr[:, soff : soff + slen], pr[:, :slen],
                        AF.Ln, bias=eps4,
                    )
                    nc.scalar.activation(
                        r32(rrow[0:4, soff : soff + slen]),
                        lnr[:, soff : soff + slen], AF.Exp, scale=-0.5,
                    )

                # ---- apply rstd + quantize ----------------------------------
                # kst blocks in the order the first slots consume them
                # (pair order hits key tiles (12,13),(2,3),(0,1),(6,7),...)
                for sb in (3, 0, 1, 2):
                    soff, slen = S_BLOCKS[sb]
                    po = psRP.tile([128, 512], F32, name="pok", tag="rp")
                    nc.tensor.matmul(
                        po[:, :slen],
                        sel_k[:, 0:128],
                        r32(rrow[0:4, soff : soff + slen]),
                    )
                    nc.vector.tensor_mul(
                        kst[:, soff : soff + slen],
                        tsk[:, soff : soff + slen],
                        po[:, :slen],
                    )
                for soff, slen in S_BLOCKS:
                    po = psRP.tile([128, 512], F32, name="po", tag="rp")
                    nc.tensor.matmul(
                        po[:, :slen],
                        sel_q[:, 0:128],
                        r32(rrow[0:4, soff : soff + slen]),
                    )
                    nc.vector.tensor_mul(
                        qst[:, soff : soff + slen],
                        tsq[:, soff : soff + slen],
                        po[:, :slen],
                    )

                # ---- qk2 fused block (evac + square for the C window) -------
                for sb in range(4):
                    qkv_block(2, sb)
                nc.gpsimd.tensor_mul(sq2[:], qk2[:], qk2[:])

                # ---- v projection: only the p6 pair tiles before phase C ----
                def vproj(j, vi, pool=None):
                    toff, tlen = T_TILES[j]
                    pool = pool or psA
                    tag = "mm" if pool is psA else "oc"
                    pt = pool.tile([128, 512], F32, name="mmv", tag=tag)
                    acc = pt[:tlen, :VP]
                    for p in range(3):
                        nc.tensor.matmul(
                            acc,
                            xraw[:, 2 * p : 2 * p + 2, toff : toff + tlen],
                            wv8s[:, 2 * p : 2 * p + 2, :],
                            start=(p == 0), stop=(p == 2), perf_mode=DR,
                        )
                    eng = (nc.vector, nc.scalar)[vi % 2]
                    if eng is nc.scalar:
                        nc.scalar.activation(
                            vx8[:tlen, j, 0:195], acc[:, 0:195], AF.Copy
                        )
                    else:
                        eng.tensor_copy(vx8[:tlen, j, 0:195], acc[:, 0:195])
                    nc.gpsimd.memset(vx8[:tlen, j, 64:195:65], 1.0)

                nc.gpsimd.memset(vx8[64:128, 13, :], 0.0)
                vproj(12, 0)
                vproj(13, 1)

            # ---- phase C: attention + interleaved ff / output ---------------
            with (
                tc.tile_pool(name="psSC", bufs=2, space="PSUM") as psSC,
                tc.tile_pool(name="psAV", bufs=2, space="PSUM") as psAV,
                tc.tile_pool(name="psFF", bufs=1, space="PSUM") as psFF,
                tc.tile_pool(name="psDG", bufs=1, space="PSUM") as psDG,
                tc.tile_pool(name="pbf", bufs=4) as pbf,
                tc.tile_pool(name="pgs", bufs=2) as pgs,
                tc.tile_pool(name="psg", bufs=2) as psg,
            ):
                def qk2_stats():
                    for soff, slen in S_BLOCKS:
                        pr = psDG.tile([2, 512], F32, name="vk2", tag="oc")
                        nc.tensor.matmul(
                            pr[:, :slen], wqk2, sq2[:, soff : soff + slen]
                        )
                        nc.scalar.activation(
                            lnr2[:, soff : soff + slen], pr[:, :slen],
                            AF.Ln, bias=eps2,
                        )
                        nc.scalar.activation(
                            r32(rrow2[:, soff : soff + slen]),
                            lnr2[:, soff : soff + slen], AF.Exp, scale=-0.5,
                        )

                def rope_qk2():
                    ts2 = pm.tile([128, S], BF16, name="ts2", tag="ts2")
                    tc2 = pm.tile([128, S], BF16, name="tc2", tag="tc2")
                    for soff, slen in S_BLOCKS:
                        rot = psDG.tile([128, 512], F32, name="rot2", tag="oc")
                        nc.tensor.matmul(
                            rot[:, :slen], rrm, qk2[:, soff : soff + slen]
                        )
                        nc.vector.tensor_mul(
                            ts2[:, soff : soff + slen],
                            rot[:, :slen],
                            sinb[:, soff : soff + slen],
                        )
                    nc.vector.tensor_mul(tc2[:], qk2[:], cosb)
                    nc.vector.tensor_add(ts2[:], ts2[:], tc2[:])
                    # rows 0-63 (q2) scaled by rstd_q2; 64-127 (k2) by rstd_k2/8
                    for soff, slen in S_BLOCKS:
                        po = psDG.tile([128, 512], F32, name="po2", tag="oc")
                        nc.tensor.matmul(
                            po[:, :slen],
                            sel_2[:, 0:128],
                            r32(rrow2[:, soff : soff + slen]),
                        )
                        nc.vector.tensor_mul(
                            q2st[:, soff : soff + slen],
                            ts2[:, soff : soff + slen],
                            po[:, :slen],
                        )
                    # align k2 to partitions 0:64 so score matmuls share a
                    # base partition with the q2 moving operand
                    nc.sync.dma_start(k2q[:, :], q2st[64:128, :])

                def ff_pair(i, fb):
                    foff, flen = FB_BLOCKS[fb]
                    of, og = 3 + i, 6 + i
                    # gate matmuls -> silu evac frees the tile -> ffx matmuls
                    # reuse it (keeps psFF at one PSUM bank)
                    pgt = psFF.tile([128, 512], F32, name="pf", tag="pf")
                    pg = pgt[:, 0:QF]
                    for p in range(3):
                        nc.tensor.matmul(
                            pg,
                            wf8s[:, 2 * p : 2 * p + 2,
                                 128 * og : 128 * (og + 1)],
                            xraw[:, 2 * p : 2 * p + 2, foff : foff + flen],
                            start=(p == 0), stop=(p == 2), perf_mode=DR,
                        )
                    gs = pgs.tile([128, QF], BF16, name="gs", tag="gs")
                    nc.scalar.activation(gs[:], pg, AF.Silu)
                    pft = psFF.tile([128, 512], F32, name="pf", tag="pf")
                    pf = pft[:, 0:QF]
                    for p in range(3):
                        nc.tensor.matmul(
                            pf,
                            wf8s[:, 2 * p : 2 * p + 2,
                                 128 * of : 128 * (of + 1)],
                            xraw[:, 2 * p : 2 * p + 2, foff : foff + flen],
                            start=(p == 0), stop=(p == 2), perf_mode=DR,
                        )
                    nc.vector.tensor_mul(
                        dff[:, i, foff : foff + flen], gs[:], pf
                    )

                def d_group(o, fb, pool=None):
                    foff, flen = FB_BLOCKS[fb]
                    acct = (pool or psDG).tile(
                        [128, 512], F32, name="oc", tag="oc"
                    )
                    acc = acct[:, 0:QF]
                    eng = (nc.vector, nc.scalar)[(o + fb) % 2]
                    nc.tensor.matmul(
                        acc, wfb[:, 0:2, 128 * o : 128 * (o + 1)],
                        dff[:, 0:2, foff : foff + flen],
                        start=True, stop=False, perf_mode=DR,
                    )
                    nc.tensor.matmul(
                        acc, wfb[:, 2:4, 128 * o : 128 * (o + 1)],
                        dff[:, 2:4, foff : foff + flen],
                        start=False, stop=False, perf_mode=DR,
                    )
                    nc.tensor.matmul(
                        acc, wa4[:, 0:2, 128 * o : 128 * (o + 1)],
                        dact[:, 0:2, foff : foff + flen],
                        start=False, stop=False, perf_mode=DR,
                    )
                    nc.tensor.matmul(
                        acc, wa4[:, 2:4, 128 * o : 128 * (o + 1)],
                        dact[:, 2:4, foff : foff + flen],
                        start=False, stop=True, perf_mode=DR,
                    )
                    if eng is nc.scalar:
                        nc.scalar.activation(
                            obr[:, o, foff : foff + flen], acc, AF.Copy
                        )
                    else:
                        eng.tensor_copy(obr[:, o, foff : foff + flen], acc)
                    if fb == 2:
                        nc.sync.dma_start(
                            outT[128 * o : 128 * (o + 1), 0 : 3 * QF],
                            obr[:, o, 0 : 3 * QF],
                        )
                    elif fb == 3:
                        nc.sync.dma_start(
                            outT[128 * o : 128 * (o + 1), 3 * QF : S],
                            obr[:, o, 3 * QF : S],
                        )

                # slot processing order: h=2 of quarter 0 deferred to 4th so
                # the qk2 rope/stats pipeline can ride works of slots 1-2
                SLOT_SEQ = [(0, 0), (0, 1), (1, 0), (0, 2), (1, 1), (1, 2),
                            (2, 0), (2, 1), (2, 2), (3, 0), (3, 1), (3, 2)]

                def vpw(js, vi):
                    def f():
                        for i, j in enumerate(js):
                            vproj(j, vi + i, psDG)
                    return f

                def ffw(i, fb):
                    return lambda: ff_pair(i, fb)

                def dgw(*ofs):
                    def f():
                        for o, fb in ofs:
                            d_group(o, fb)
                    return f

                # per-slot extra work, emitted interleaved with attention;
                # slot 0 emits the remaining v tiles just-in-time for its own
                # A*V consumption order (12,13),(2,3),(0,1),(6,7),(4,5),...
                slot_work = {
                    (0, 0): [vpw((2, 3, 0, 1), 2), vpw((6, 7, 4, 5), 6),
                             vpw((10, 11, 8, 9), 10)],
                    (0, 1): [qk2_stats, ffw(0, 0), ffw(1, 0)],
                    (1, 0): [rope_qk2, ffw(2, 0), ffw(0, 1)],
                    (0, 2): [ffw(1, 1), ffw(2, 1)],
                    (1, 1): [ffw(0, 2), dgw((0, 0)), dgw((1, 0))],
                    (1, 2): [ffw(1, 2), dgw((2, 0)), dgw((3, 0))],
                    (2, 0): [ffw(2, 2), dgw((4, 0)), dgw((5, 0))],
                    (2, 1): [ffw(0, 3), dgw((0, 1)), dgw((1, 1))],
                    (2, 2): [ffw(1, 3), dgw((2, 1)), dgw((3, 1))],
                    (3, 0): [ffw(2, 3), dgw((4, 1)), dgw((5, 1))],
                    (3, 1): [dgw((0, 2), (1, 2)), dgw((2, 2)), dgw((3, 2))],
                    (3, 2): [dgw((4, 2)), dgw((5, 2))],
                }

                pb13d = [
                    pm.tile([128, 2, QF], F8, name=f"pb13_{i}", tag=f"pb13_{i}")
                    for i in range(2)
                ]
                for i in range(2):
                    nc.gpsimd.memset(pb13d[i][64:128, 1, :], 0.0)

                def emit_av(av, pos, p, pb, h):
                    j0 = 2 * p
                    nc.tensor.matmul(
                        av[:, :],
                        vx8[:, j0 : j0 + 2, 65 * h : 65 * h + 65],
                        pb[:, :, :],
                        start=(pos == 0), stop=(pos == 6),
                        perf_mode=DR,
                    )

                for sidx, (qf, h) in enumerate(SLOT_SEQ):
                        qoff = QF * qf
                        works = list(slot_work.get((qf, h), ()))
                        # odd slots push pair 5 to ACT to balance engine load
                        dvp = DVE_PAIRS if sidx % 2 == 0 else (1, 3)
                        av = psAV.tile([65, QF], F32, name="av", tag="av")
                        prev = None
                        for pos, p in enumerate(PAIR_ORDER):
                            j0, j1 = 2 * p, 2 * p + 1
                            on_dve = p in dvp
                            pb = pb13d[sidx % 2] if p == 6 else pbf.tile(
                                [128, 2, QF], F8, name="pbf", tag="pbf"
                            )
                            if h == 2:
                                kt_src, qt_src = k2q, q2st
                                kr0, qr0 = 0, 0
                            else:
                                kt_src, qt_src = kst, qst
                                kr0 = qr0 = 64 * h
                            scp = psSC.tile([128, 2, 512], F32,
                                            name="scp", tag="scp")
                            for jj, j in enumerate((j0, j1)):
                                toff, tlen = T_TILES[j]
                                nc.tensor.matmul(
                                    scp[:tlen, jj, 0:QF],
                                    kt_src[kr0 : kr0 + 64, toff : toff + tlen],
                                    qt_src[qr0 : qr0 + 64, qoff : qoff + QF],
                                )
                            if on_dve:
                                nc.vector.tensor_scalar(
                                    pb[:, :, :].bitcast(U8),
                                    scp[:, 0:2, 0:QF],
                                    ACOEF8, BCONST8, ALU.mult, ALU.add,
                                )
                            elif p == 6:
                                nc.scalar.activation(
                                    pb[:, 0, :], scp[:, 0, 0:QF],
                                    AF.Exp, bias=cm8[:],
                                )
                                nc.scalar.activation(
                                    pb[0:64, 1, :], scp[0:64, 1, 0:QF],
                                    AF.Exp, bias=cm8[0:64, :],
                                )
                            else:
                                nc.scalar.activation(
                                    pb[:, :, :], scp[:, 0:2, 0:QF],
                                    AF.Exp, bias=cm8[:],
                                )
                            if prev is not None:
                                emit_av(av, *prev, h)
                            if pos in WORK_AT and works:
                                works.pop(0)()
                            prev = (pos, p, pb)
                        emit_av(av, *prev, h)
                        segs = psg.tile([1, QF], F32, name="segs", tag="segs")
                        with nc.allow_low_precision(
                            reason="f32r denominators feed a broadcast matmul"
                        ):
                            nc.vector.reciprocal(r32(segs[:]), av[64:65, :])
                        pobt = psDG.tile([128, 512], F32, name="pob", tag="oc")
                        pob = pobt[0:64, 0:QF]
                        nc.tensor.matmul(pob, e164, r32(segs[:]))
                        nc.vector.tensor_mul(
                            dact[:, h, qoff : qoff + QF], av[0:64, :], pob
                        )
                        for work in works:
                            work()

            # tail: last-quarter output groups get a fresh deep PSUM pool
            with tc.tile_pool(name="psT", bufs=3, space="PSUM") as psT:
                for o in range(6):
                    d_group(o, 3, psT)

    _split_excess_waits(nc)
    return nc


# ---------------------------------------------------------------------------
# host-side preparation
# ---------------------------------------------------------------------------


def _axial_freqs():
    base = np.linspace(1.0, MAX_FREQ / 2, 8) * math.pi

    def ax(n):
        pos = np.linspace(-1.0, 1.0, n)
        return np.repeat(pos[:, None] * base[None, :], 2, axis=-1)

    fH = np.broadcast_to(ax(H)[:, None, None, :], (H, W, D, 16))
    fW = np.broadcast_to(ax(W)[None, :, None, :], (H, W, D, 16))
    fD = np.broadcast_to(ax(D)[None, None, :, :], (H, W, D, 16))
    return np.concatenate((fH, fW, fD), axis=-1).reshape(S, ROT)


def _chunked(mat):
    """[768, C] -> [128, 6, C] (chunk-major rows to partition-major)."""
    C = mat.shape[1]
    return np.ascontiguousarray(mat.reshape(6, 128, C).transpose(1, 0, 2))


def _prep_core_inputs(x, norm1_w, w_fused, b_fused, q_gamma, q_beta, k_gamma,
                      k_beta, w_attn, w_ff, b_ff):
    """Returns list of 8 in_maps (core = b*4 + r)."""
    f64 = np.float64
    F8NP = mybir.dt.np(F8)
    BF16NP = mybir.dt.np(BF16)
    w_fused = np.asarray(w_fused, f64)
    q_gamma = np.asarray(q_gamma, f64)
    k_gamma = np.asarray(k_gamma, f64)

    if np.any(np.asarray(b_fused)) or np.any(np.asarray(b_ff)):
        raise NotImplementedError("nonzero biases not supported by this kernel")
    if np.any(np.asarray(q_beta)) or np.any(np.asarray(k_beta)):
        raise NotImplementedError("nonzero q/k beta not supported by this kernel")
    if np.any(q_gamma == 0) or np.any(k_gamma == 0):
        raise NotImplementedError("zero gamma not supported by this kernel")

    M = np.eye(HD) - np.ones((HD, HD)) / HD
    Aq = np.diag(q_gamma) @ M
    Ak = np.diag(k_gamma) @ M
    R = np.zeros((HD, HD))
    for i in range(ROT // 2):
        R[2 * i, 2 * i + 1] = -1.0
        R[2 * i + 1, 2 * i] = 1.0
    R2 = np.zeros((128, 128))
    R2[0:64, 0:64] = R
    R2[64:128, 64:128] = R

    freqs = _axial_freqs()
    cos64 = np.ones((HD, S))
    sin64 = np.zeros((HD, S))
    cos64[:ROT, :] = np.cos(freqs).T
    sin64[:ROT, :] = np.sin(freqs).T
    cosT = np.vstack([cos64, cos64])
    sinT = np.vstack([sin64, sin64])

    packR = np.zeros((128, PR_COLS))
    packR[:, PR_RR[0] : PR_RR[1]] = R2.T
    packR[:, PR_COS[0] : PR_COS[1]] = cosT
    packR[:, PR_SIN[0] : PR_SIN[1]] = sinT
    packR = packR.astype(BF16NP)

    wq_full = w_fused[MLP : MLP + HID]
    wk_full = w_fused[MLP + HID : MLP + 2 * HID]
    wv_full = w_fused[MLP + 2 * HID :]
    ffx_full = w_fused[: MLP // 2]
    gate_full = w_fused[MLP // 2 : MLP]

    nw = np.asarray(norm1_w, np.float32).reshape(6, 128).T
    iq = 1.0 / (HD * q_gamma**2)
    ik = 1.0 / k_gamma**2
    wq01 = np.zeros((128, 2))
    wq01[0:64, 0] = iq
    wq01[64:128, 1] = iq
    wk01 = np.zeros((128, 2))
    wk01[0:64, 0] = ik
    wk01[64:128, 1] = ik
    wqk2 = np.zeros((128, 2))
    wqk2[0:64, 0] = iq
    wqk2[64:128, 1] = ik

    packS = np.zeros((128, PS_COLS), np.float32)
    packS[:, PS_NW[0] : PS_NW[1]] = nw
    packS[0, PS_SELQ[0] : PS_SELQ[0] + 64] = 1.0
    packS[1, PS_SELQ[0] + 64 : PS_SELQ[0] + 128] = 1.0
    packS[2, PS_SELK[0] : PS_SELK[0] + 64] = 1.0
    packS[3, PS_SELK[0] + 64 : PS_SELK[0] + 128] = 1.0
    packS[0, PS_SEL2[0] : PS_SEL2[0] + 64] = 1.0
    packS[1, PS_SEL2[0] + 64 : PS_SEL2[0] + 128] = 1.0
    packS[0, PS_E164[0] : PS_E164[1]] = 1.0
    packS[0:4, PS_EPS4[0]] = [EPS_LN, EPS_LN, 64 * EPS_LN, 64 * EPS_LN]
    packS[0:2, PS_EPS2[0]] = [EPS_LN, 64 * EPS_LN]

    packB = np.zeros((128, PB_COLS))
    packB[:, PB_WQ4[0] : PB_WQ4[0] + 2] = wq01
    packB[:, PB_WK4[0] + 2 : PB_WK4[0] + 4] = wk01
    packB[:, PB_WQK2[0] : PB_WQK2[1]] = wqk2
    packB = packB.astype(BF16NP)

    w_attn = np.asarray(w_attn, f64)
    w_ff = np.asarray(w_ff, f64)
    in_maps = []
    for core in range(N_CORES):
        b, r = divmod(core, TP)
        hs = [HPC * r + i for i in range(HPC)]
        q3 = [Aq @ wq_full[HD * h : HD * (h + 1)] for h in hs]
        k3 = [Ak @ wk_full[HD * h : HD * (h + 1)] for h in hs]
        ffx = ffx_full[FFPC * r : FFPC * (r + 1)]
        gate = gate_full[FFPC * r : FFPC * (r + 1)]
        wf_mat = np.vstack(
            [q3[0], q3[1], k3[0], k3[1], q3[2], k3[2], ffx, gate]
        ).T  # [HID, NF]
        wv_mat = np.zeros((VP, HID))
        for i, h in enumerate(hs):
            wv_mat[65 * i : 65 * i + HD] = wv_full[HD * h : HD * (h + 1)]
        wa4_np = np.zeros((64, 4, HID))
        wa4_np[:, 0, :] = w_attn[:, HD * hs[0] : HD * hs[0] + HD].T
        wa4_np[:, 1, :] = w_attn[:, HD * hs[1] : HD * hs[1] + HD].T
        wa4_np[:, 2, :] = w_attn[:, HD * hs[2] : HD * hs[2] + HD].T
        wffr = w_ff[:, FFPC * r : FFPC * (r + 1)]
        wfb_np = np.zeros((128, 4, HID))
        wfb_np[:, 0, :] = wffr[:, 0:128].T
        wfb_np[:, 1, :] = wffr[:, 128:256].T
        wfb_np[:, 2, :] = wffr[:, 256:384].T
        in_maps.append(
            {
                "xT": _chunked(
                    np.asarray(x[b], np.float32).reshape(HID, S)
                ).astype(F8NP),
                "wfT": _chunked(wf_mat).astype(BF16NP),
                "wvT": _chunked(wv_mat.T).astype(BF16NP),
                "wa4T": wa4_np.astype(F8NP),
                "wfbT": wfb_np.astype(F8NP),
                "packST": packS,
                "packBT": packB,
                "packRT": packR,
            }
        )
    return in_maps


_NC_CACHE = {}


def get_program():
    if "nc" not in _NC_CACHE:
        _NC_CACHE["nc"] = build_program()
    return _NC_CACHE["nc"]


def kernel(**inputs) -> np.ndarray:
    nc = get_program()
    in_maps = _prep_core_inputs(**inputs)
    res = bass_utils.run_bass_kernel_spmd(nc, in_maps, core_ids=list(range(N_CORES)))
    out = np.zeros((B, HID, H, W, D), np.float32)
    for core in range(N_CORES):
        b = core // TP
        out[b] += res.results[core]["outT"].astype(np.float32).reshape(
            HID, H, W, D
        )
    out += np.asarray(inputs["x"], np.float32)
    return out


# revision 47
# speedup vs baseline: 1.0546x; 1.0546x over previous
"""Trainium2 Bass kernel for nn_FullAttention_17789754540074.

Self-contained: takes the FULL inputs of reference.setup_inputs(), returns the
FULL output. Internally shards across 8 NeuronCores as 2-way data parallel
(batch) x 4-way tensor parallel (3 heads + 384 FF pairs per rank), runs one
SPMD Bass/Tile program via run_bass_kernel_spmd, and sums the 4 partial
outputs per batch on the host, adding the residual there too (the unshard
step for partial-sum TP sharding).

v2 structure (vs the 161us baseline):
  - x ships fp8; the RMS-norm scale is folded into the fused/v WEIGHTS
    (1152+208 cols per chunk on the scale pass instead of 1728), so the
    QKV/ff/v matmuls consume raw fp8 x directly
  - inputs arrive in 15 DMAs (constant packs by dtype + priority) instead of
    47; outputs leave in 12 per-o row DMAs instead of 24 quarter DMAs
  - softmax: all score pairs land in [128,2,512] PSUM tiles; ACT pairs use a
    single merged exp per pair; DVE pairs use a uint8 Schraudolph writing
    fp8e4 probs directly (saturates at 0 below, bits<=126 above by XBIAS
    choice), so ALL A*V matmuls run fp8 DoubleRow and the bf16 v copy dies
  - per-slot normalize fused: dact = av[0:64] * pob with both operands in
    PSUM (no separate att evacuation)
  - rope cos/sin muls run bf16 2x on DVE instead of Pool
"""

import math

import numpy as np

import concourse.bass as bass
import concourse.mybir as mybir
import concourse.tile as tile
from concourse import bass_utils
from concourse.vector_clock import ScopedClock

F32 = mybir.dt.float32
F32R = mybir.dt.float32r
F8 = mybir.dt.float8e4
BF16 = mybir.dt.bfloat16
U8 = mybir.dt.uint8
AF = mybir.ActivationFunctionType
ALU = mybir.AluOpType
DR = mybir.MatmulPerfMode.DoubleRow

HID, HEADS, HD, MLP = 768, 12, 64, 3072
B, H, W, D = 2, 12, 12, 12
S = H * W * D  # 1728
ROT = 48
MAX_FREQ = 256.0
EPS_GN, EPS_LN = 1e-6, 1e-5

N_CORES = 8
TP = 4
HPC = 3  # heads per core
FFPC = 384  # ff pairs per core
NF = 9 * 128  # fused rows: q01 | k01 | q2k2 | ffx*3 | gate*3
VP = 208  # v proj cols: [v0|1|v1|1|v2|1] = 195 used + pad

S_BLOCKS = [(0, 512), (512, 512), (1024, 448), (1472, 256)]
T_TILES = [(128 * j, 128) for j in range(13)] + [(1664, 64)]
QF = S // 4  # 432 queries per attention slot
FB_BLOCKS = [(QF * q, QF) for q in range(4)]
TR_BLOCKS = [(0, 512), (512, 512), (1024, 512), (1536, 192)]

# Softmax exp bias: keeps Schraudolph uint8 bits <= ~120 (below the 0x7F NaN
# encoding) for scores up to ~8, while bits<0 saturate to 0 (prob 0), roughly
# matching the fp8 subnormal flush of the ACT exp path.
XBIAS = -2.5
# fp8e4m3 Schraudolph: u8 = sc*ACOEF8 + BCONST8; bits(u8) ~ e4m3(exp(sc+XBIAS))
ACOEF8 = float(8.0 / math.log(2.0))
BCONST8 = float(8.0 * (XBIAS / math.log(2.0) + 7.0) - 0.490)
# which key-tile pairs per slot run on DVE (rest on ACT)
DVE_PAIRS = (1, 3, 5)
# pair emission order within a slot: alternate ACT/DVE consumers, start with
# the odd p=6 pair so the slot tail ends on fast pairs
PAIR_ORDER = (6, 1, 0, 3, 2, 5, 4)
# emit interleaved work at these POSITIONS in PAIR_ORDER (the DVE pairs, so
# the works' ACT/PE load lands while ACT is otherwise idle)
WORK_AT = (1, 3, 5)


# constant-pack column layout (see _prep_core_inputs)
PS_NW = (0, 6)
PS_SELQ = (6, 134)
PS_SELK = (134, 262)
PS_SEL2 = (262, 390)
PS_E164 = (390, 454)
PS_EPS4 = (454, 455)  # rows: EPS, EPS, 64*EPS, 64*EPS
PS_EPS2 = (455, 456)  # rows: EPS, 64*EPS
PS_COLS = 456
PB_WQ4 = (0, 4)  # cols (iq_q0, iq_q1, 0, 0)
PB_WK4 = (4, 8)  # cols (0, 0, ik_k0, ik_k1)
PB_WQK2 = (8, 10)
PB_COLS = 10
PR_RR = (0, 128)
PR_COS = (128, 128 + S)
PR_SIN = (128 + S, 128 + 2 * S)
PR_COLS = 128 + 2 * S


class TileContextSplitDrain(tile.TileContext):
    """TileContext whose kernel-tail drain splits its semaphore waits across
    single-wait sync NOPs — the walrus build here rejects >2 sync waits on one
    SP CTRL instruction ("Too many sync wait commands")."""

    def _drain_and_barrier(self, tick_clock, wait_clock):
        probe = self.nc.sync.nop(nofuse=True)
        wait_clock.add_sem_waits(
            probe.ins, ScopedClock({None: tick_clock.global_clock})
        )
        si = probe.ins.sync_info
        waits = list(si.on_wait) if si is not None else []
        if si is not None:
            si.on_wait = waits[:1]
        for w in waits[1:]:
            n = self.nc.sync.nop(nofuse=True)
            nsi = n.ins.sync_info
            if nsi is None:
                n.ins.sync_info = mybir.SyncInfo(on_wait=[w], on_update=[])
            else:
                nsi.on_wait.append(w)
        self.nc.sync.drain()
        self.nc.all_engine_barrier()
        popped = self.nc._tile_sem_poison_stack.pop()
        assert popped is self._sem_poison
        self.nc.clear_and_free_semaphores(list(self.sems.allocated().values()))
        self.nc.all_engine_barrier()


def r32(ap):
    return ap.bitcast(F32R)


def _split_excess_waits(nc, maxw=1):
    """walrus in this container caps sync waits per instruction; move extras
    onto preceding same-engine NOPs (waits execute in program order)."""
    nid = 0
    for bb in nc.m.functions[0].blocks:
        insts = bb.instructions
        i = 0
        while i < len(insts):
            inst = insts[i]
            si = inst.sync_info
            nw = len(si.on_wait) if si is not None and si.on_wait else 0
            if nw > maxw:
                waits = list(si.on_wait)
                si.on_wait = waits[-maxw:]
                extra = waits[:-maxw]
                pos = i
                for k in range(0, len(extra), maxw):
                    nop = mybir.InstNoOp(
                        name=f"I-waitsplit-{nid}", ins=[], outs=[]
                    )
                    nop.engine = inst.engine
                    nop.sync_info = mybir.SyncInfo(
                        on_wait=extra[k : k + maxw], on_update=[]
                    )
                    insts.insert(pos, nop)
                    nc.register_instruction(nop)
                    pos += 1
                    i += 1
                    nid += 1
            i += 1


def build_program():
    nc = bass.Bass(trn_type="TRN2")

    xT = nc.dram_tensor("xT", [128, 6, S], F8, kind="ExternalInput")
    wfT = nc.dram_tensor("wfT", [128, 6, NF], BF16, kind="ExternalInput")
    wvT = nc.dram_tensor("wvT", [128, 6, VP], BF16, kind="ExternalInput")
    wa4T = nc.dram_tensor("wa4T", [64, 4, HID], F8, kind="ExternalInput")
    wfbT = nc.dram_tensor("wfbT", [128, 4, HID], F8, kind="ExternalInput")
    packST = nc.dram_tensor("packST", [128, PS_COLS], F32, kind="ExternalInput")
    packBT = nc.dram_tensor("packBT", [128, PB_COLS], BF16, kind="ExternalInput")
    packRT = nc.dram_tensor("packRT", [128, PR_COLS], BF16, kind="ExternalInput")
    outT = nc.dram_tensor("outT", [HID, S], BF16, kind="ExternalOutput")

    with TileContextSplitDrain(nc) as tc:
        with tc.tile_pool(name="main", bufs=1) as pm:
            # ---- long-lived SBUF tiles --------------------------------------
            xraw = pm.tile([128, 6, S], F8, name="xraw", tag="xraw")
            wf16 = pm.tile([128, 6, NF], BF16, name="wf16", tag="wf16")
            wv16 = pm.tile([128, 6, VP], BF16, name="wv16", tag="wv16")
            wf8s = pm.tile([128, 6, NF], F8, name="wf8s", tag="wf8s")
            wv8s = pm.tile([128, 6, VP], F8, name="wv8s", tag="wv8s")
            wa4 = pm.tile([64, 4, HID], F8, name="wa4", tag="wa4")
            wfb = pm.tile([128, 4, HID], F8, name="wfb", tag="wfb")
            packS = pm.tile([128, PS_COLS], F32, name="packS", tag="packS")
            packB = pm.tile([128, PB_COLS], BF16, name="packB", tag="packB")
            packR = pm.tile([128, PR_COLS], BF16, name="packR", tag="packR")
            qab = pm.tile([128, S], BF16, name="qab", tag="qab")
            kab = pm.tile([128, S], BF16, name="kab", tag="kab")
            qk2 = pm.tile([128, S], BF16, name="qk2", tag="qk2")
            qst = pm.tile([128, S], F8, name="qst", tag="qst")
            kst = pm.tile([128, S], F8, name="kst", tag="kst")
            q2st = pm.tile([128, S], F8, name="q2st", tag="q2st")
            k2q = pm.tile([64, S], F8, name="k2q", tag="k2q")
            vx8 = pm.tile([128, 14, VP], F8, name="vx8", tag="vx8")
            dact = pm.tile([HD, 4, S], F8, name="dact", tag="dact")
            dff = pm.tile([128, 4, S], F8, name="dff", tag="dff")
            sqq = pm.tile([128, S], BF16, name="sqq", tag="sqq")
            sqk = pm.tile([128, S], BF16, name="sqk", tag="sqk")
            sq2 = pm.tile([128, S], BF16, name="sq2", tag="sq2")
            sqju = pm.tile([128, 2, S], F8, name="sqju", tag="sqju")
            lnr = pm.tile([4, S], F32, name="lnr", tag="lnr")
            lnr2 = pm.tile([2, S], F32, name="lnr2", tag="lnr2")
            rrow = pm.tile([4, S], F32, name="rrow", tag="rrow")
            rrow2 = pm.tile([2, S], F32, name="rrow2", tag="rrow2")
            ss12 = pm.tile([128, 6], F32, name="ss12", tag="ss12")
            rmsc = pm.tile([128, 6], F32, name="rmsc", tag="rmsc")
            scl6 = pm.tile([128, 6], F32, name="scl6", tag="scl6")
            cgn = pm.tile([128, 1], F32, name="cgn", tag="cgn")
            cm8 = pm.tile([128, 1], F32, name="cm8", tag="cm8")
            tsq = pm.tile([128, S], BF16, name="tsq", tag="tsq")
            tcq = pm.tile([128, S], BF16, name="tcq", tag="tcq")
            tsk = pm.tile([128, S], BF16, name="tsk", tag="tsk")
            tck = pm.tile([128, S], BF16, name="tck", tag="tck")
            obr = pm.tile([128, 6, S], BF16, name="obr", tag="obr")

            # constant-pack slices
            nw = packS[:, PS_NW[0] : PS_NW[1]]
            sel_q = r32(packS[0:4, PS_SELQ[0] : PS_SELQ[1]])
            sel_k = r32(packS[0:4, PS_SELK[0] : PS_SELK[1]])
            sel_2 = r32(packS[0:2, PS_SEL2[0] : PS_SEL2[1]])
            e164 = r32(packS[0:1, PS_E164[0] : PS_E164[1]])
            wq4 = packB[:, PB_WQ4[0] : PB_WQ4[1]]
            wk4 = packB[:, PB_WK4[0] : PB_WK4[1]]
            wqk2 = packB[:, PB_WQK2[0] : PB_WQK2[1]]
            eps4 = packS[0:4, PS_EPS4[0] : PS_EPS4[1]]
            eps2 = packS[0:2, PS_EPS2[0] : PS_EPS2[1]]
            rrm = packR[:, PR_RR[0] : PR_RR[1]]
            cosb = packR[:, PR_COS[0] : PR_COS[1]]
            sinb = packR[:, PR_SIN[0] : PR_SIN[1]]

            # ---- input DMAs (priority order) --------------------------------
            nc.sync.dma_start(packS[:], packST[:])
            for c in range(6):
                nc.sync.dma_start(xraw[:, c, :], xT[:, c, :])
            nc.sync.dma_start(packB[:], packBT[:])
            for p in range(3):
                nc.sync.dma_start(
                    wf16[:, 2 * p : 2 * p + 2, :], wfT[:, 2 * p : 2 * p + 2, :]
                )
            nc.sync.dma_start(packR[:], packRT[:])
            nc.sync.dma_start(wv16[:], wvT[:])
            nc.sync.dma_start(wa4[:], wa4T[:])
            nc.sync.dma_start(wfb[:], wfbT[:])

            # ---- constants / zero pads --------------------------------------
            nc.vector.memset(cgn[:], EPS_GN)
            nc.vector.memset(cm8[:], XBIAS)

            nc.gpsimd.memset(dact[:, 3, :], 0.0)
            nc.gpsimd.memset(dff[:, 3, :], 0.0)


            # ---- phase A: rms norm stats + weight scaling + fused qkv -------
            with (
                tc.tile_pool(name="psA", bufs=3, space="PSUM") as psA,
                tc.tile_pool(name="psRP", bufs=3, space="PSUM") as psRP,
            ):
                for c in range(6):
                    nc.scalar.activation(
                        sqju[:, c % 2, :], xraw[:, c, :], AF.Square,
                        accum_out=ss12[:, c : c + 1],
                    )
                    nc.scalar.activation(
                        rmsc[:, c : c + 1], ss12[:, c : c + 1], AF.Sqrt,
                        bias=cgn[:], scale=1.0 / S,
                    )
                    nc.vector.reciprocal(
                        scl6[:, c : c + 1], rmsc[:, c : c + 1]
                    )
                    nc.vector.tensor_mul(
                        scl6[:, c : c + 1], scl6[:, c : c + 1], nw[:, c : c + 1]
                    )
                    # scale weights (not x): wf8s = fp8(wf16 * scl), ditto wv
                    weng = (nc.scalar, nc.vector, nc.scalar,
                            nc.vector, nc.scalar, nc.vector)[c]
                    if weng is nc.scalar:
                        nc.scalar.activation(
                            wf8s[:, c, :], wf16[:, c, :], AF.Copy,
                            scale=scl6[:, c : c + 1],
                        )
                    else:
                        weng.tensor_scalar(
                            wf8s[:, c, :], wf16[:, c, :],
                            scl6[:, c : c + 1], None, ALU.mult,
                        )
                    nc.gpsimd.tensor_scalar(
                        wv8s[:, c, :], wv16[:, c, :],
                        scl6[:, c : c + 1], None, ALU.mult,
                    )

                # qkv q01/k01 blocks first (o=0,1), evacs spread over engines
                qk_dst = [qab, kab, qk2]

                def qkv_block(o, sb):
                    soff, slen = S_BLOCKS[sb]
                    pt = psA.tile([128, 512], F32, name="mm", tag="mm")
                    acc = pt[:, :slen]
                    for p in range(3):
                        nc.tensor.matmul(
                            acc,
                            wf8s[:, 2 * p : 2 * p + 2,
                                 128 * o : 128 * (o + 1)],
                            xraw[:, 2 * p : 2 * p + 2, soff : soff + slen],
                            start=(p == 0), stop=(p == 2), perf_mode=DR,
                        )
                    eng = (nc.vector, nc.scalar)[(o + sb) % 2]
                    if eng is nc.scalar:
                        nc.scalar.activation(
                            qk_dst[o][:, soff : soff + slen], acc, AF.Copy
                        )
                    else:
                        eng.tensor_copy(qk_dst[o][:, soff : soff + slen], acc)

                for o in range(2):
                    for sb in range(4):
                        qkv_block(o, sb)

                # ---- rope sin/cos parts (no stats dependency) ---------------
                # per-block: sin-mul on DVE (reads PSUM), cos-mul on Pool,
                # add on DVE (bf16 2x) — q and k chains overlap; the squares
                # for the token stats ride per-block on ACT
                for src, ts_t, tc_t, sq_t in (
                    (qab, tsq, tcq, sqq), (kab, tsk, tck, sqk)
                ):
                    for soff, slen in S_BLOCKS:
                        rot = psRP.tile([128, 512], F32, name="rot", tag="rp")
                        nc.tensor.matmul(
                            rot[:, :slen], rrm, src[:, soff : soff + slen]
                        )
                        nc.gpsimd.tensor_mul(
                            tc_t[:, soff : soff + slen],
                            src[:, soff : soff + slen],
                            cosb[:, soff : soff + slen],
                        )
                        nc.scalar.activation(
                            sq_t[:, soff : soff + slen],
                            src[:, soff : soff + slen], AF.Square,
                        )
                        nc.vector.tensor_mul(
                            ts_t[:, soff : soff + slen],
                            rot[:, :slen],
                            sinb[:, soff : soff + slen],
                        )
                        nc.vector.tensor_add(
                            ts_t[:, soff : soff + slen],
                            ts_t[:, soff : soff + slen],
                            tc_t[:, soff : soff + slen],
                        )

                # ---- q01/k01 rstd stats, directly in row layout -------------
                # per-token variance sums land as [4, block] rows via
                # moving=squares matmuls; rstd = exp(-0.5 * ln(sum + eps))
                for soff, slen in S_BLOCKS:
                    pr = psA.tile([4, 512], F32, name="vk", tag="mm")
                    nc.tensor.matmul(
                        pr[:, :slen], wq4, sqq[:, soff : soff + slen],
                        start=True, stop=False,
                    )
                    nc.tensor.matmul(
                        pr[:, :slen], wk4, sqk[:, soff : soff + slen],
                        start=False, stop=True,
                    )
                    nc.scalar.activation(
                        lnr[:, soff : soff + slen], pr[:, :slen],
                        AF.Ln, bias=eps4,
                    )
                    nc.scalar.activation(
                        r32(rrow[0:4, soff : soff + slen]),
                        lnr[:, soff : soff + slen], AF.Exp, scale=-0.5,
                    )

                # ---- apply rstd + quantize ----------------------------------
                # kst blocks in the order the first slots consume them
                # (pair order hits key tiles (12,13),(2,3),(0,1),(6,7),...)
                for sb in (3, 0, 1, 2):
                    soff, slen = S_BLOCKS[sb]
                    po = psRP.tile([128, 512], F32, name="pok", tag="rp")
                    nc.tensor.matmul(
                        po[:, :slen],
                        sel_k[:, 0:128],
                        r32(rrow[0:4, soff : soff + slen]),
                    )
                    nc.vector.tensor_mul(
                        kst[:, soff : soff + slen],
                        tsk[:, soff : soff + slen],
                        po[:, :slen],
                    )
                for soff, slen in S_BLOCKS:
                    po = psRP.tile([128, 512], F32, name="po", tag="rp")
                    nc.tensor.matmul(
                        po[:, :slen],
                        sel_q[:, 0:128],
                        r32(rrow[0:4, soff : soff + slen]),
                    )
                    nc.vector.tensor_mul(
                        qst[:, soff : soff + slen],
                        tsq[:, soff : soff + slen],
                        po[:, :slen],
                    )

                # ---- qk2 fused block (evac + square for the C window) -------
                for sb in range(4):
                    qkv_block(2, sb)
                nc.gpsimd.tensor_mul(sq2[:], qk2[:], qk2[:])

                # ---- v projection: only the p6 pair tiles before phase C ----
                def vproj(j, vi, pool=None):
                    toff, tlen = T_TILES[j]
                    pool = pool or psA
                    tag = "mm" if pool is psA else "oc"
                    pt = pool.tile([128, 512], F32, name="mmv", tag=tag)
                    acc = pt[:tlen, :VP]
                    for p in range(3):
                        nc.tensor.matmul(
                            acc,
                            xraw[:, 2 * p : 2 * p + 2, toff : toff + tlen],
                            wv8s[:, 2 * p : 2 * p + 2, :],
                            start=(p == 0), stop=(p == 2), perf_mode=DR,
                        )
                    eng = (nc.vector, nc.scalar)[vi % 2]
                    if eng is nc.scalar:
                        nc.scalar.activation(
                            vx8[:tlen, j, 0:195], acc[:, 0:195], AF.Copy
                        )
                    else:
                        eng.tensor_copy(vx8[:tlen, j, 0:195], acc[:, 0:195])
                    nc.gpsimd.memset(vx8[:tlen, j, 64:195:65], 1.0)

                nc.gpsimd.memset(vx8[64:128, 13, :], 0.0)
                vproj(12, 0)
                vproj(13, 1)

            # ---- phase C: attention + interleaved ff / output ---------------
            with (
                tc.tile_pool(name="psSC", bufs=4, space="PSUM") as psSC,
                tc.tile_pool(name="psAV", bufs=2, space="PSUM") as psAV,
                tc.tile_pool(name="psFF", bufs=1, space="PSUM") as psFF,
                tc.tile_pool(name="psDG", bufs=1, space="PSUM") as psDG,
                tc.tile_pool(name="pbf", bufs=4) as pbf,
                tc.tile_pool(name="pgs", bufs=2) as pgs,
                tc.tile_pool(name="psg", bufs=2) as psg,
            ):
                def qk2_stats():
                    for soff, slen in S_BLOCKS:
                        pr = psDG.tile([2, 512], F32, name="vk2", tag="oc")
                        nc.tensor.matmul(
                            pr[:, :slen], wqk2, sq2[:, soff : soff + slen]
                        )
                        nc.scalar.activation(
                            lnr2[:, soff : soff + slen], pr[:, :slen],
                            AF.Ln, bias=eps2,
                        )
                        nc.scalar.activation(
                            r32(rrow2[:, soff : soff + slen]),
                            lnr2[:, soff : soff + slen], AF.Exp, scale=-0.5,
                        )

                def rope_qk2():
                    ts2 = pm.tile([128, S], BF16, name="ts2", tag="ts2")
                    tc2 = pm.tile([128, S], BF16, name="tc2", tag="tc2")
                    for soff, slen in S_BLOCKS:
                        rot = psDG.tile([128, 512], F32, name="rot2", tag="oc")
                        nc.tensor.matmul(
                            rot[:, :slen], rrm, qk2[:, soff : soff + slen]
                        )
                        nc.vector.tensor_mul(
                            ts2[:, soff : soff + slen],
                            rot[:, :slen],
                            sinb[:, soff : soff + slen],
                        )
                    nc.vector.tensor_mul(tc2[:], qk2[:], cosb)
                    nc.vector.tensor_add(ts2[:], ts2[:], tc2[:])
                    # rows 0-63 (q2) scaled by rstd_q2; 64-127 (k2) by rstd_k2/8
                    for soff, slen in S_BLOCKS:
                        po = psDG.tile([128, 512], F32, name="po2", tag="oc")
                        nc.tensor.matmul(
                            po[:, :slen],
                            sel_2[:, 0:128],
                            r32(rrow2[:, soff : soff + slen]),
                        )
                        nc.vector.tensor_mul(
                            q2st[:, soff : soff + slen],
                            ts2[:, soff : soff + slen],
                            po[:, :slen],
                        )
                    # align k2 to partitions 0:64 so score matmuls share a
                    # base partition with the q2 moving operand
                    nc.sync.dma_start(k2q[:, :], q2st[64:128, :])

                def ff_pair(i, fb):
                    foff, flen = FB_BLOCKS[fb]
                    of, og = 3 + i, 6 + i
                    # gate matmuls -> silu evac frees the tile -> ffx matmuls
                    # reuse it (keeps psFF at one PSUM bank)
                    pgt = psFF.tile([128, 512], F32, name="pf", tag="pf")
                    pg = pgt[:, 0:QF]
                    for p in range(3):
                        nc.tensor.matmul(
                            pg,
                            wf8s[:, 2 * p : 2 * p + 2,
                                 128 * og : 128 * (og + 1)],
                            xraw[:, 2 * p : 2 * p + 2, foff : foff + flen],
                            start=(p == 0), stop=(p == 2), perf_mode=DR,
                        )
                    gs = pgs.tile([128, QF], BF16, name="gs", tag="gs")
                    nc.scalar.activation(gs[:], pg, AF.Silu)
                    pft = psFF.tile([128, 512], F32, name="pf", tag="pf")
                    pf = pft[:, 0:QF]
                    for p in range(3):
                        nc.tensor.matmul(
                            pf,
                            wf8s[:, 2 * p : 2 * p + 2,
                                 128 * of : 128 * (of + 1)],
                            xraw[:, 2 * p : 2 * p + 2, foff : foff + flen],
                            start=(p == 0), stop=(p == 2), perf_mode=DR,
                        )
                    nc.vector.tensor_mul(
                        dff[:, i, foff : foff + flen], gs[:], pf
                    )

                def d_group(o, fb, pool=None):
                    foff, flen = FB_BLOCKS[fb]
                    acct = (pool or psDG).tile(
                        [128, 512], F32, name="oc", tag="oc"
                    )
                    acc = acct[:, 0:QF]
                    eng = (nc.vector, nc.scalar)[(o + fb) % 2]
                    nc.tensor.matmul(
                        acc, wfb[:, 0:2, 128 * o : 128 * (o + 1)],
                        dff[:, 0:2, foff : foff + flen],
                        start=True, stop=False, perf_mode=DR,
                    )
                    nc.tensor.matmul(
                        acc, wfb[:, 2:4, 128 * o : 128 * (o + 1)],
                        dff[:, 2:4, foff : foff + flen],
                        start=False, stop=False, perf_mode=DR,
                    )
                    nc.tensor.matmul(
                        acc, wa4[:, 0:2, 128 * o : 128 * (o + 1)],
                        dact[:, 0:2, foff : foff + flen],
                        start=False, stop=False, perf_mode=DR,
                    )
                    nc.tensor.matmul(
                        acc, wa4[:, 2:4, 128 * o : 128 * (o + 1)],
                        dact[:, 2:4, foff : foff + flen],
                        start=False, stop=True, perf_mode=DR,
                    )
                    if eng is nc.scalar:
                        nc.scalar.activation(
                            obr[:, o, foff : foff + flen], acc, AF.Copy
                        )
                    else:
                        eng.tensor_copy(obr[:, o, foff : foff + flen], acc)
                    if fb == 2:
                        nc.sync.dma_start(
                            outT[128 * o : 128 * (o + 1), 0 : 3 * QF],
                            obr[:, o, 0 : 3 * QF],
                        )
                    elif fb == 3:
                        nc.sync.dma_start(
                            outT[128 * o : 128 * (o + 1), 3 * QF : S],
                            obr[:, o, 3 * QF : S],
                        )

                # slot processing order: h=2 of quarter 0 deferred to 4th so
                # the qk2 rope/stats pipeline can ride works of slots 1-2
                SLOT_SEQ = [(0, 0), (0, 1), (1, 0), (0, 2), (1, 1), (1, 2),
                            (2, 0), (2, 1), (2, 2), (3, 0), (3, 1), (3, 2)]

                def vpw(js, vi):
                    def f():
                        for i, j in enumerate(js):
                            vproj(j, vi + i, psDG)
                    return f

                def ffw(i, fb):
                    return lambda: ff_pair(i, fb)

                def dgw(*ofs):
                    def f():
                        for o, fb in ofs:
                            d_group(o, fb)
                    return f

                # per-slot extra work, emitted interleaved with attention;
                # slot 0 emits the remaining v tiles just-in-time for its own
                # A*V consumption order (12,13),(2,3),(0,1),(6,7),(4,5),...
                slot_work = {
                    (0, 0): [vpw((2, 3, 0, 1), 2), vpw((6, 7, 4, 5), 6),
                             vpw((10, 11, 8, 9), 10)],
                    (0, 1): [qk2_stats, ffw(0, 0), ffw(1, 0)],
                    (1, 0): [rope_qk2, ffw(2, 0), ffw(0, 1)],
                    (0, 2): [ffw(1, 1), ffw(2, 1)],
                    (1, 1): [ffw(0, 2), dgw((0, 0)), dgw((1, 0))],
                    (1, 2): [ffw(1, 2), dgw((2, 0)), dgw((3, 0))],
                    (2, 0): [ffw(2, 2), dgw((4, 0)), dgw((5, 0))],
                    (2, 1): [ffw(0, 3), dgw((0, 1)), dgw((1, 1))],
                    (2, 2): [ffw(1, 3), dgw((2, 1)), dgw((3, 1))],
                    (3, 0): [ffw(2, 3), dgw((4, 1)), dgw((5, 1))],
                    (3, 1): [dgw((0, 2), (1, 2)), dgw((2, 2)), dgw((3, 2))],
                    (3, 2): [dgw((4, 2)), dgw((5, 2))],
                }

                pb13d = [
                    pm.tile([128, 2, QF], F8, name=f"pb13_{i}", tag=f"pb13_{i}")
                    for i in range(2)
                ]
                for i in range(2):
                    nc.gpsimd.memset(pb13d[i][64:128, 1, :], 0.0)

                def emit_av(av, pos, p, pb, h):
                    j0 = 2 * p
                    nc.tensor.matmul(
                        av[:, :],
                        vx8[:, j0 : j0 + 2, 65 * h : 65 * h + 65],
                        pb[:, :, :],
                        start=(pos == 0), stop=(pos == 6),
                        perf_mode=DR,
                    )

                for sidx, (qf, h) in enumerate(SLOT_SEQ):
                        qoff = QF * qf
                        works = list(slot_work.get((qf, h), ()))
                        av = psAV.tile([65, QF], F32, name="av", tag="av")
                        prev = None
                        for pos, p in enumerate(PAIR_ORDER):
                            j0, j1 = 2 * p, 2 * p + 1
                            pb = pb13d[sidx % 2] if p == 6 else pbf.tile(
                                [128, 2, QF], F8, name="pbf", tag="pbf"
                            )
                            if h == 2:
                                kt_src, qt_src = k2q, q2st
                                kr0, qr0 = 0, 0
                            else:
                                kt_src, qt_src = kst, qst
                                kr0 = qr0 = 64 * h
                            for jj, j in enumerate((j0, j1)):
                                toff, tlen = T_TILES[j]
                                sc = psSC.tile([128, 512], F32,
                                               name="sc", tag="sc")
                                nc.tensor.matmul(
                                    sc[:tlen, 0:QF],
                                    kt_src[kr0 : kr0 + 64, toff : toff + tlen],
                                    qt_src[qr0 : qr0 + 64, qoff : qoff + QF],
                                )
                                if (2 * pos + jj + sidx) % 2 == 0:
                                    nc.scalar.activation(
                                        pb[:tlen, jj, :], sc[:tlen, 0:QF],
                                        AF.Exp, bias=cm8[:tlen, :],
                                    )
                                else:
                                    nc.vector.tensor_scalar(
                                        pb[:tlen, jj, :].bitcast(U8),
                                        sc[:tlen, 0:QF],
                                        ACOEF8, BCONST8, ALU.mult, ALU.add,
                                    )
                            if prev is not None:
                                emit_av(av, *prev, h)
                            if pos in WORK_AT and works:
                                works.pop(0)()
                            prev = (pos, p, pb)
                        emit_av(av, *prev, h)
                        segs = psg.tile([1, QF], F32, name="segs", tag="segs")
                        with nc.allow_low_precision(
                            reason="f32r denominators feed a broadcast matmul"
                        ):
                            nc.vector.reciprocal(r32(segs[:]), av[64:65, :])
                        pobt = psDG.tile([128, 512], F32, name="pob", tag="oc")
                        pob = pobt[0:64, 0:QF]
                        nc.tensor.matmul(pob, e164, r32(segs[:]))
                        nc.vector.tensor_mul(
                            dact[:, h, qoff : qoff + QF], av[0:64, :], pob
                        )
                        for work in works:
                            work()

            # tail: last-quarter output groups get a fresh deep PSUM pool
            with tc.tile_pool(name="psT", bufs=3, space="PSUM") as psT:
                for o in range(6):
                    d_group(o, 3, psT)

    _split_excess_waits(nc)
    return nc


# ---------------------------------------------------------------------------
# host-side preparation
# ---------------------------------------------------------------------------


def _axial_freqs():
    base = np.linspace(1.0, MAX_FREQ / 2, 8) * math.pi

    def ax(n):
        pos = np.linspace(-1.0, 1.0, n)
        return np.repeat(pos[:, None] * base[None, :], 2, axis=-1)

    fH = np.broadcast_to(ax(H)[:, None, None, :], (H, W, D, 16))
    fW = np.broadcast_to(ax(W)[None, :, None, :], (H, W, D, 16))
    fD = np.broadcast_to(ax(D)[None, None, :, :], (H, W, D, 16))
    return np.concatenate((fH, fW, fD), axis=-1).reshape(S, ROT)


def _chunked(mat):
    """[768, C] -> [128, 6, C] (chunk-major rows to partition-major)."""
    C = mat.shape[1]
    return np.ascontiguousarray(mat.reshape(6, 128, C).transpose(1, 0, 2))


def _prep_core_inputs(x, norm1_w, w_fused, b_fused, q_gamma, q_beta, k_gamma,
                      k_beta, w_attn, w_ff, b_ff):
    """Returns list of 8 in_maps (core = b*4 + r)."""
    f64 = np.float64
    F8NP = mybir.dt.np(F8)
    BF16NP = mybir.dt.np(BF16)
    w_fused = np.asarray(w_fused, f64)
    q_gamma = np.asarray(q_gamma, f64)
    k_gamma = np.asarray(k_gamma, f64)

    if np.any(np.asarray(b_fused)) or np.any(np.asarray(b_ff)):
        raise NotImplementedError("nonzero biases not supported by this kernel")
    if np.any(np.asarray(q_beta)) or np.any(np.asarray(k_beta)):
        raise NotImplementedError("nonzero q/k beta not supported by this kernel")
    if np.any(q_gamma == 0) or np.any(k_gamma == 0):
        raise NotImplementedError("zero gamma not supported by this kernel")

    M = np.eye(HD) - np.ones((HD, HD)) / HD
    Aq = np.diag(q_gamma) @ M
    Ak = np.diag(k_gamma) @ M
    R = np.zeros((HD, HD))
    for i in range(ROT // 2):
        R[2 * i, 2 * i + 1] = -1.0
        R[2 * i + 1, 2 * i] = 1.0
    R2 = np.zeros((128, 128))
    R2[0:64, 0:64] = R
    R2[64:128, 64:128] = R

    freqs = _axial_freqs()
    cos64 = np.ones((HD, S))
    sin64 = np.zeros((HD, S))
    cos64[:ROT, :] = np.cos(freqs).T
    sin64[:ROT, :] = np.sin(freqs).T
    cosT = np.vstack([cos64, cos64])
    sinT = np.vstack([sin64, sin64])

    packR = np.zeros((128, PR_COLS))
    packR[:, PR_RR[0] : PR_RR[1]] = R2.T
    packR[:, PR_COS[0] : PR_COS[1]] = cosT
    packR[:, PR_SIN[0] : PR_SIN[1]] = sinT
    packR = packR.astype(BF16NP)

    wq_full = w_fused[MLP : MLP + HID]
    wk_full = w_fused[MLP + HID : MLP + 2 * HID]
    wv_full = w_fused[MLP + 2 * HID :]
    ffx_full = w_fused[: MLP // 2]
    gate_full = w_fused[MLP // 2 : MLP]

    nw = np.asarray(norm1_w, np.float32).reshape(6, 128).T
    iq = 1.0 / (HD * q_gamma**2)
    ik = 1.0 / k_gamma**2
    wq01 = np.zeros((128, 2))
    wq01[0:64, 0] = iq
    wq01[64:128, 1] = iq
    wk01 = np.zeros((128, 2))
    wk01[0:64, 0] = ik
    wk01[64:128, 1] = ik
    wqk2 = np.zeros((128, 2))
    wqk2[0:64, 0] = iq
    wqk2[64:128, 1] = ik

    packS = np.zeros((128, PS_COLS), np.float32)
    packS[:, PS_NW[0] : PS_NW[1]] = nw
    packS[0, PS_SELQ[0] : PS_SELQ[0] + 64] = 1.0
    packS[1, PS_SELQ[0] + 64 : PS_SELQ[0] + 128] = 1.0
    packS[2, PS_SELK[0] : PS_SELK[0] + 64] = 1.0
    packS[3, PS_SELK[0] + 64 : PS_SELK[0] + 128] = 1.0
    packS[0, PS_SEL2[0] : PS_SEL2[0] + 64] = 1.0
    packS[1, PS_SEL2[0] + 64 : PS_SEL2[0] + 128] = 1.0
    packS[0, PS_E164[0] : PS_E164[1]] = 1.0
    packS[0:4, PS_EPS4[0]] = [EPS_LN, EPS_LN, 64 * EPS_LN, 64 * EPS_LN]
    packS[0:2, PS_EPS2[0]] = [EPS_LN, 64 * EPS_LN]

    packB = np.zeros((128, PB_COLS))
    packB[:, PB_WQ4[0] : PB_WQ4[0] + 2] = wq01
    packB[:, PB_WK4[0] + 2 : PB_WK4[0] + 4] = wk01
    packB[:, PB_WQK2[0] : PB_WQK2[1]] = wqk2
    packB = packB.astype(BF16NP)

    w_attn = np.asarray(w_attn, f64)
    w_ff = np.asarray(w_ff, f64)
    in_maps = []
    for core in range(N_CORES):
        b, r = divmod(core, TP)
        hs = [HPC * r + i for i in range(HPC)]
        q3 = [Aq @ wq_full[HD * h : HD * (h + 1)] for h in hs]
        k3 = [Ak @ wk_full[HD * h : HD * (h + 1)] for h in hs]
        ffx = ffx_full[FFPC * r : FFPC * (r + 1)]
        gate = gate_full[FFPC * r : FFPC * (r + 1)]
        wf_mat = np.vstack(
            [q3[0], q3[1], k3[0], k3[1], q3[2], k3[2], ffx, gate]
        ).T  # [HID, NF]
        wv_mat = np.zeros((VP, HID))
        for i, h in enumerate(hs):
            wv_mat[65 * i : 65 * i + HD] = wv_full[HD * h : HD * (h + 1)]
        wa4_np = np.zeros((64, 4, HID))
        wa4_np[:, 0, :] = w_attn[:, HD * hs[0] : HD * hs[0] + HD].T
        wa4_np[:, 1, :] = w_attn[:, HD * hs[1] : HD * hs[1] + HD].T
        wa4_np[:, 2, :] = w_attn[:, HD * hs[2] : HD * hs[2] + HD].T
        wffr = w_ff[:, FFPC * r : FFPC * (r + 1)]
        wfb_np = np.zeros((128, 4, HID))
        wfb_np[:, 0, :] = wffr[:, 0:128].T
        wfb_np[:, 1, :] = wffr[:, 128:256].T
        wfb_np[:, 2, :] = wffr[:, 256:384].T
        in_maps.append(
            {
                "xT": _chunked(
                    np.asarray(x[b], np.float32).reshape(HID, S)
                ).astype(F8NP),
                "wfT": _chunked(wf_mat).astype(BF16NP),
                "wvT": _chunked(wv_mat.T).astype(BF16NP),
                "wa4T": wa4_np.astype(F8NP),
                "wfbT": wfb_np.astype(F8NP),
                "packST": packS,
                "packBT": packB,
                "packRT": packR,
            }
        )
    return in_maps


_NC_CACHE = {}


def get_program():
    if "nc" not in _NC_CACHE:
        _NC_CACHE["nc"] = build_program()
    return _NC_CACHE["nc"]


def kernel(**inputs) -> np.ndarray:
    nc = get_program()
    in_maps = _prep_core_inputs(**inputs)
    res = bass_utils.run_bass_kernel_spmd(nc, in_maps, core_ids=list(range(N_CORES)))
    out = np.zeros((B, HID, H, W, D), np.float32)
    for core in range(N_CORES):
        b = core // TP
        out[b] += res.results[core]["outT"].astype(np.float32).reshape(
            HID, H, W, D
        )
    out += np.asarray(inputs["x"], np.float32)
    return out


# revision 49
# speedup vs baseline: 1.0793x; 1.0234x over previous
"""Trainium2 Bass kernel for nn_FullAttention_17789754540074.

Self-contained: takes the FULL inputs of reference.setup_inputs(), returns the
FULL output. Internally shards across 8 NeuronCores as 2-way data parallel
(batch) x 4-way tensor parallel (3 heads + 384 FF pairs per rank), runs one
SPMD Bass/Tile program via run_bass_kernel_spmd, and sums the 4 partial
outputs per batch on the host, adding the residual there too (the unshard
step for partial-sum TP sharding).

v2 structure (vs the 161us baseline):
  - x ships fp8; the RMS-norm scale is folded into the fused/v WEIGHTS
    (1152+208 cols per chunk on the scale pass instead of 1728), so the
    QKV/ff/v matmuls consume raw fp8 x directly
  - inputs arrive in 15 DMAs (constant packs by dtype + priority) instead of
    47; outputs leave in 12 per-o row DMAs instead of 24 quarter DMAs
  - softmax: all score pairs land in [128,2,512] PSUM tiles; ACT pairs use a
    single merged exp per pair; DVE pairs use a uint8 Schraudolph writing
    fp8e4 probs directly (saturates at 0 below, bits<=126 above by XBIAS
    choice), so ALL A*V matmuls run fp8 DoubleRow and the bf16 v copy dies
  - per-slot normalize fused: dact = av[0:64] * pob with both operands in
    PSUM (no separate att evacuation)
  - rope cos/sin muls run bf16 2x on DVE instead of Pool
"""

import math

import numpy as np

import concourse.bass as bass
import concourse.mybir as mybir
import concourse.tile as tile
from concourse import bass_utils
from concourse.vector_clock import ScopedClock

F32 = mybir.dt.float32
F32R = mybir.dt.float32r
F8 = mybir.dt.float8e4
BF16 = mybir.dt.bfloat16
U8 = mybir.dt.uint8
AF = mybir.ActivationFunctionType
ALU = mybir.AluOpType
DR = mybir.MatmulPerfMode.DoubleRow

HID, HEADS, HD, MLP = 768, 12, 64, 3072
B, H, W, D = 2, 12, 12, 12
S = H * W * D  # 1728
ROT = 48
MAX_FREQ = 256.0
EPS_GN, EPS_LN = 1e-6, 1e-5

N_CORES = 8
TP = 4
HPC = 3  # heads per core
FFPC = 384  # ff pairs per core
NF = 9 * 128  # fused rows: q01 | k01 | q2k2 | ffx*3 | gate*3
VP = 208  # v proj cols: [v0|1|v1|1|v2|1] = 195 used + pad

S_BLOCKS = [(0, 512), (512, 512), (1024, 448), (1472, 256)]
T_TILES = [(128 * j, 128) for j in range(13)] + [(1664, 64)]
QF = S // 4  # 432 queries per attention slot
FB_BLOCKS = [(QF * q, QF) for q in range(4)]
TR_BLOCKS = [(0, 512), (512, 512), (1024, 512), (1536, 192)]

# Softmax exp bias: keeps Schraudolph uint8 bits <= ~120 (below the 0x7F NaN
# encoding) for scores up to ~8, while bits<0 saturate to 0 (prob 0), roughly
# matching the fp8 subnormal flush of the ACT exp path.
XBIAS = -2.5
# fp8e4m3 Schraudolph: u8 = sc*ACOEF8 + BCONST8; bits(u8) ~ e4m3(exp(sc+XBIAS))
ACOEF8 = float(8.0 / math.log(2.0))
BCONST8 = float(8.0 * (XBIAS / math.log(2.0) + 7.0) - 0.490)
# which key-tile pairs per slot run on DVE (rest on ACT)
DVE_PAIRS = (1, 3, 5)
# pair emission order within a slot: alternate ACT/DVE consumers, start with
# the odd p=6 pair so the slot tail ends on fast pairs
PAIR_ORDER = (6, 1, 0, 3, 2, 5, 4)
# emit interleaved work at these POSITIONS in PAIR_ORDER (the DVE pairs, so
# the works' ACT/PE load lands while ACT is otherwise idle)
WORK_AT = (1, 3, 5)


# constant-pack column layout (see _prep_core_inputs)
PS_NW = (0, 6)
PS_SELQ = (6, 134)
PS_SELK = (134, 262)
PS_SEL2 = (262, 390)
PS_E164 = (390, 454)
PS_EPS4 = (454, 455)  # rows: EPS, EPS, 64*EPS, 64*EPS
PS_EPS2 = (455, 456)  # rows: EPS, 64*EPS
PS_COLS = 456
PB_WQ4 = (0, 4)  # cols (iq_q0, iq_q1, 0, 0)
PB_WK4 = (4, 8)  # cols (0, 0, ik_k0, ik_k1)
PB_WQK2 = (8, 10)
PB_COLS = 10
PR_RR = (0, 128)
PR_COS = (128, 128 + S)
PR_SIN = (128 + S, 128 + 2 * S)
PR_COLS = 128 + 2 * S


class TileContextSplitDrain(tile.TileContext):
    """TileContext whose kernel-tail drain splits its semaphore waits across
    single-wait sync NOPs — the walrus build here rejects >2 sync waits on one
    SP CTRL instruction ("Too many sync wait commands")."""

    def _drain_and_barrier(self, tick_clock, wait_clock):
        probe = self.nc.sync.nop(nofuse=True)
        wait_clock.add_sem_waits(
            probe.ins, ScopedClock({None: tick_clock.global_clock})
        )
        si = probe.ins.sync_info
        waits = list(si.on_wait) if si is not None else []
        if si is not None:
            si.on_wait = waits[:1]
        for w in waits[1:]:
            n = self.nc.sync.nop(nofuse=True)
            nsi = n.ins.sync_info
            if nsi is None:
                n.ins.sync_info = mybir.SyncInfo(on_wait=[w], on_update=[])
            else:
                nsi.on_wait.append(w)
        self.nc.sync.drain()
        self.nc.all_engine_barrier()
        popped = self.nc._tile_sem_poison_stack.pop()
        assert popped is self._sem_poison
        self.nc.clear_and_free_semaphores(list(self.sems.allocated().values()))
        self.nc.all_engine_barrier()


def r32(ap):
    return ap.bitcast(F32R)


def _split_excess_waits(nc, maxw=1):
    """walrus in this container caps sync waits per instruction; move extras
    onto preceding same-engine NOPs (waits execute in program order)."""
    nid = 0
    for bb in nc.m.functions[0].blocks:
        insts = bb.instructions
        i = 0
        while i < len(insts):
            inst = insts[i]
            si = inst.sync_info
            nw = len(si.on_wait) if si is not None and si.on_wait else 0
            if nw > maxw:
                waits = list(si.on_wait)
                si.on_wait = waits[-maxw:]
                extra = waits[:-maxw]
                pos = i
                for k in range(0, len(extra), maxw):
                    nop = mybir.InstNoOp(
                        name=f"I-waitsplit-{nid}", ins=[], outs=[]
                    )
                    nop.engine = inst.engine
                    nop.sync_info = mybir.SyncInfo(
                        on_wait=extra[k : k + maxw], on_update=[]
                    )
                    insts.insert(pos, nop)
                    nc.register_instruction(nop)
                    pos += 1
                    i += 1
                    nid += 1
            i += 1


def build_program():
    nc = bass.Bass(trn_type="TRN2")

    xT = nc.dram_tensor("xT", [128, 6, S], F8, kind="ExternalInput")
    wfT = nc.dram_tensor("wfT", [128, 6, NF], BF16, kind="ExternalInput")
    wvT = nc.dram_tensor("wvT", [128, 6, VP], BF16, kind="ExternalInput")
    wa4T = nc.dram_tensor("wa4T", [64, 4, HID], F8, kind="ExternalInput")
    wfbT = nc.dram_tensor("wfbT", [128, 4, HID], F8, kind="ExternalInput")
    packST = nc.dram_tensor("packST", [128, PS_COLS], F32, kind="ExternalInput")
    packBT = nc.dram_tensor("packBT", [128, PB_COLS], BF16, kind="ExternalInput")
    packRT = nc.dram_tensor("packRT", [128, PR_COLS], BF16, kind="ExternalInput")
    outT = nc.dram_tensor("outT", [HID, S], BF16, kind="ExternalOutput")

    with TileContextSplitDrain(nc) as tc:
        with tc.tile_pool(name="main", bufs=1) as pm:
            # ---- long-lived SBUF tiles --------------------------------------
            xraw = pm.tile([128, 6, S], F8, name="xraw", tag="xraw")
            wf16 = pm.tile([128, 6, NF], BF16, name="wf16", tag="wf16")
            wv16 = pm.tile([128, 6, VP], BF16, name="wv16", tag="wv16")
            wf8s = pm.tile([128, 6, NF], F8, name="wf8s", tag="wf8s")
            wv8s = pm.tile([128, 6, VP], F8, name="wv8s", tag="wv8s")
            wa4 = pm.tile([64, 4, HID], F8, name="wa4", tag="wa4")
            wfb = pm.tile([128, 4, HID], F8, name="wfb", tag="wfb")
            packS = pm.tile([128, PS_COLS], F32, name="packS", tag="packS")
            packB = pm.tile([128, PB_COLS], BF16, name="packB", tag="packB")
            packR = pm.tile([128, PR_COLS], BF16, name="packR", tag="packR")
            qab = pm.tile([128, S], BF16, name="qab", tag="qab")
            kab = pm.tile([128, S], BF16, name="kab", tag="kab")
            qk2 = pm.tile([128, S], BF16, name="qk2", tag="qk2")
            qst = pm.tile([128, S], F8, name="qst", tag="qst")
            kst = pm.tile([128, S], F8, name="kst", tag="kst")
            q2st = pm.tile([128, S], F8, name="q2st", tag="q2st")
            k2q = pm.tile([64, S], F8, name="k2q", tag="k2q")
            vx8 = pm.tile([128, 14, VP], F8, name="vx8", tag="vx8")
            dact = pm.tile([HD, 4, S], F8, name="dact", tag="dact")
            dff = pm.tile([128, 4, S], F8, name="dff", tag="dff")
            sqq = pm.tile([128, S], BF16, name="sqq", tag="sqq")
            sqk = pm.tile([128, S], BF16, name="sqk", tag="sqk")
            sq2 = pm.tile([128, S], BF16, name="sq2", tag="sq2")
            sqju = pm.tile([128, 2, S], F8, name="sqju", tag="sqju")
            lnr = pm.tile([4, S], F32, name="lnr", tag="lnr")
            lnr2 = pm.tile([2, S], F32, name="lnr2", tag="lnr2")
            rrow = pm.tile([4, S], F32, name="rrow", tag="rrow")
            rrow2 = pm.tile([2, S], F32, name="rrow2", tag="rrow2")
            ss12 = pm.tile([128, 6], F32, name="ss12", tag="ss12")
            rmsc = pm.tile([128, 6], F32, name="rmsc", tag="rmsc")
            scl6 = pm.tile([128, 6], F32, name="scl6", tag="scl6")
            cgn = pm.tile([128, 1], F32, name="cgn", tag="cgn")
            cm8 = pm.tile([128, 1], F32, name="cm8", tag="cm8")
            tsq = pm.tile([128, S], BF16, name="tsq", tag="tsq")
            tcq = pm.tile([128, S], BF16, name="tcq", tag="tcq")
            tsk = pm.tile([128, S], BF16, name="tsk", tag="tsk")
            tck = pm.tile([128, S], BF16, name="tck", tag="tck")
            obr = pm.tile([128, 6, S], BF16, name="obr", tag="obr")

            # constant-pack slices
            nw = packS[:, PS_NW[0] : PS_NW[1]]
            sel_q = r32(packS[0:4, PS_SELQ[0] : PS_SELQ[1]])
            sel_k = r32(packS[0:4, PS_SELK[0] : PS_SELK[1]])
            sel_2 = r32(packS[0:2, PS_SEL2[0] : PS_SEL2[1]])
            e164 = r32(packS[0:1, PS_E164[0] : PS_E164[1]])
            wq4 = packB[:, PB_WQ4[0] : PB_WQ4[1]]
            wk4 = packB[:, PB_WK4[0] : PB_WK4[1]]
            wqk2 = packB[:, PB_WQK2[0] : PB_WQK2[1]]
            eps4 = packS[0:4, PS_EPS4[0] : PS_EPS4[1]]
            eps2 = packS[0:2, PS_EPS2[0] : PS_EPS2[1]]
            rrm = packR[:, PR_RR[0] : PR_RR[1]]
            cosb = packR[:, PR_COS[0] : PR_COS[1]]
            sinb = packR[:, PR_SIN[0] : PR_SIN[1]]

            # ---- input DMAs (priority order) --------------------------------
            nc.sync.dma_start(packS[:], packST[:])
            for c in range(6):
                nc.sync.dma_start(xraw[:, c, :], xT[:, c, :])
            nc.sync.dma_start(packB[:], packBT[:])
            for p in range(3):
                nc.sync.dma_start(
                    wf16[:, 2 * p : 2 * p + 2, :], wfT[:, 2 * p : 2 * p + 2, :]
                )
            nc.sync.dma_start(packR[:], packRT[:])
            nc.sync.dma_start(wv16[:], wvT[:])
            nc.sync.dma_start(wa4[:], wa4T[:])
            nc.sync.dma_start(wfb[:], wfbT[:])

            # ---- constants / zero pads --------------------------------------
            nc.vector.memset(cgn[:], EPS_GN)
            nc.vector.memset(cm8[:], XBIAS)

            nc.gpsimd.memset(dact[:, 3, :], 0.0)
            nc.gpsimd.memset(dff[:, 3, :], 0.0)


            # ---- phase A: rms norm stats + weight scaling + fused qkv -------
            with (
                tc.tile_pool(name="psA", bufs=3, space="PSUM") as psA,
                tc.tile_pool(name="psRP", bufs=3, space="PSUM") as psRP,
            ):
                for c in range(6):
                    nc.scalar.activation(
                        sqju[:, c % 2, :], xraw[:, c, :], AF.Square,
                        accum_out=ss12[:, c : c + 1],
                    )
                    nc.scalar.activation(
                        rmsc[:, c : c + 1], ss12[:, c : c + 1], AF.Sqrt,
                        bias=cgn[:], scale=1.0 / S,
                    )
                    nc.vector.reciprocal(
                        scl6[:, c : c + 1], rmsc[:, c : c + 1]
                    )
                    nc.vector.tensor_mul(
                        scl6[:, c : c + 1], scl6[:, c : c + 1], nw[:, c : c + 1]
                    )
                    # scale weights (not x): wf8s = fp8(wf16 * scl), ditto wv
                    weng = (nc.scalar, nc.vector, nc.scalar,
                            nc.vector, nc.scalar, nc.vector)[c]
                    if weng is nc.scalar:
                        nc.scalar.activation(
                            wf8s[:, c, :], wf16[:, c, :], AF.Copy,
                            scale=scl6[:, c : c + 1],
                        )
                    else:
                        weng.tensor_scalar(
                            wf8s[:, c, :], wf16[:, c, :],
                            scl6[:, c : c + 1], None, ALU.mult,
                        )
                    nc.gpsimd.tensor_scalar(
                        wv8s[:, c, :], wv16[:, c, :],
                        scl6[:, c : c + 1], None, ALU.mult,
                    )

                # qkv q01/k01 blocks first (o=0,1), evacs spread over engines
                qk_dst = [qab, kab, qk2]

                def qkv_block(o, sb):
                    soff, slen = S_BLOCKS[sb]
                    pt = psA.tile([128, 512], F32, name="mm", tag="mm")
                    acc = pt[:, :slen]
                    for p in range(3):
                        nc.tensor.matmul(
                            acc,
                            wf8s[:, 2 * p : 2 * p + 2,
                                 128 * o : 128 * (o + 1)],
                            xraw[:, 2 * p : 2 * p + 2, soff : soff + slen],
                            start=(p == 0), stop=(p == 2), perf_mode=DR,
                        )
                    eng = (nc.vector, nc.scalar)[(o + sb) % 2]
                    if eng is nc.scalar:
                        nc.scalar.activation(
                            qk_dst[o][:, soff : soff + slen], acc, AF.Copy
                        )
                    else:
                        eng.tensor_copy(qk_dst[o][:, soff : soff + slen], acc)

                for o in range(2):
                    for sb in range(4):
                        qkv_block(o, sb)

                # ---- rope sin/cos parts (no stats dependency) ---------------
                # per-block: sin-mul on DVE (reads PSUM), cos-mul on Pool,
                # add on DVE (bf16 2x) — q and k chains overlap; the squares
                # for the token stats ride per-block on ACT
                for src, ts_t, tc_t, sq_t in (
                    (qab, tsq, tcq, sqq), (kab, tsk, tck, sqk)
                ):
                    for soff, slen in S_BLOCKS:
                        rot = psRP.tile([128, 512], F32, name="rot", tag="rp")
                        nc.tensor.matmul(
                            rot[:, :slen], rrm, src[:, soff : soff + slen]
                        )
                        nc.gpsimd.tensor_mul(
                            tc_t[:, soff : soff + slen],
                            src[:, soff : soff + slen],
                            cosb[:, soff : soff + slen],
                        )
                        nc.scalar.activation(
                            sq_t[:, soff : soff + slen],
                            src[:, soff : soff + slen], AF.Square,
                        )
                        nc.vector.tensor_mul(
                            ts_t[:, soff : soff + slen],
                            rot[:, :slen],
                            sinb[:, soff : soff + slen],
                        )
                        nc.vector.tensor_add(
                            ts_t[:, soff : soff + slen],
                            ts_t[:, soff : soff + slen],
                            tc_t[:, soff : soff + slen],
                        )

                # ---- q01/k01 rstd stats, directly in row layout -------------
                # per-token variance sums land as [4, block] rows via
                # moving=squares matmuls; rstd = exp(-0.5 * ln(sum + eps))
                for soff, slen in S_BLOCKS:
                    pr = psA.tile([4, 512], F32, name="vk", tag="mm")
                    nc.tensor.matmul(
                        pr[:, :slen], wq4, sqq[:, soff : soff + slen],
                        start=True, stop=False,
                    )
                    nc.tensor.matmul(
                        pr[:, :slen], wk4, sqk[:, soff : soff + slen],
                        start=False, stop=True,
                    )
                    nc.scalar.activation(
                        lnr[:, soff : soff + slen], pr[:, :slen],
                        AF.Ln, bias=eps4,
                    )
                    nc.scalar.activation(
                        r32(rrow[0:4, soff : soff + slen]),
                        lnr[:, soff : soff + slen], AF.Exp, scale=-0.5,
                    )

                # ---- apply rstd + quantize ----------------------------------
                # kst blocks in the order the first slots consume them
                # (pair order hits key tiles (12,13),(2,3),(0,1),(6,7),...)
                for sb in (3, 0, 1, 2):
                    soff, slen = S_BLOCKS[sb]
                    po = psRP.tile([128, 512], F32, name="pok", tag="rp")
                    nc.tensor.matmul(
                        po[:, :slen],
                        sel_k[:, 0:128],
                        r32(rrow[0:4, soff : soff + slen]),
                    )
                    nc.vector.tensor_mul(
                        kst[:, soff : soff + slen],
                        tsk[:, soff : soff + slen],
                        po[:, :slen],
                    )
                for soff, slen in S_BLOCKS:
                    po = psRP.tile([128, 512], F32, name="po", tag="rp")
                    nc.tensor.matmul(
                        po[:, :slen],
                        sel_q[:, 0:128],
                        r32(rrow[0:4, soff : soff + slen]),
                    )
                    nc.vector.tensor_mul(
                        qst[:, soff : soff + slen],
                        tsq[:, soff : soff + slen],
                        po[:, :slen],
                    )

                # ---- qk2 fused block (evac + square for the C window) -------
                for sb in range(4):
                    qkv_block(2, sb)
                nc.gpsimd.tensor_mul(sq2[:], qk2[:], qk2[:])

                # ---- v projection: only the p6 pair tiles before phase C ----
                def vproj(j, vi, pool=None):
                    toff, tlen = T_TILES[j]
                    pool = pool or psA
                    tag = "mm" if pool is psA else "oc"
                    pt = pool.tile([128, 512], F32, name="mmv", tag=tag)
                    acc = pt[:tlen, :VP]
                    for p in range(3):
                        nc.tensor.matmul(
                            acc,
                            xraw[:, 2 * p : 2 * p + 2, toff : toff + tlen],
                            wv8s[:, 2 * p : 2 * p + 2, :],
                            start=(p == 0), stop=(p == 2), perf_mode=DR,
                        )
                    eng = (nc.vector, nc.scalar)[vi % 2]
                    if eng is nc.scalar:
                        nc.scalar.activation(
                            vx8[:tlen, j, 0:195], acc[:, 0:195], AF.Copy
                        )
                    else:
                        eng.tensor_copy(vx8[:tlen, j, 0:195], acc[:, 0:195])
                    nc.gpsimd.memset(vx8[:tlen, j, 64:195:65], 1.0)

                nc.gpsimd.memset(vx8[64:128, 13, :], 0.0)
                vproj(12, 0)
                vproj(13, 1)

            # ---- phase C: attention + interleaved ff / output ---------------
            with (
                tc.tile_pool(name="psSC", bufs=4, space="PSUM") as psSC,
                tc.tile_pool(name="psAV", bufs=2, space="PSUM") as psAV,
                tc.tile_pool(name="psFF", bufs=1, space="PSUM") as psFF,
                tc.tile_pool(name="psDG", bufs=1, space="PSUM") as psDG,
                tc.tile_pool(name="pbf", bufs=4) as pbf,
                tc.tile_pool(name="pgs", bufs=2) as pgs,
                tc.tile_pool(name="psg", bufs=2) as psg,
            ):
                def qk2_stats():
                    for soff, slen in S_BLOCKS:
                        pr = psDG.tile([2, 512], F32, name="vk2", tag="oc")
                        nc.tensor.matmul(
                            pr[:, :slen], wqk2, sq2[:, soff : soff + slen]
                        )
                        nc.scalar.activation(
                            lnr2[:, soff : soff + slen], pr[:, :slen],
                            AF.Ln, bias=eps2,
                        )
                        nc.scalar.activation(
                            r32(rrow2[:, soff : soff + slen]),
                            lnr2[:, soff : soff + slen], AF.Exp, scale=-0.5,
                        )

                def rope_qk2():
                    ts2 = pm.tile([128, S], BF16, name="ts2", tag="ts2")
                    tc2 = pm.tile([128, S], BF16, name="tc2", tag="tc2")
                    for soff, slen in S_BLOCKS:
                        rot = psDG.tile([128, 512], F32, name="rot2", tag="oc")
                        nc.tensor.matmul(
                            rot[:, :slen], rrm, qk2[:, soff : soff + slen]
                        )
                        nc.vector.tensor_mul(
                            ts2[:, soff : soff + slen],
                            rot[:, :slen],
                            sinb[:, soff : soff + slen],
                        )
                    nc.vector.tensor_mul(tc2[:], qk2[:], cosb)
                    nc.vector.tensor_add(ts2[:], ts2[:], tc2[:])
                    # rows 0-63 (q2) scaled by rstd_q2; 64-127 (k2) by rstd_k2/8
                    for soff, slen in S_BLOCKS:
                        po = psDG.tile([128, 512], F32, name="po2", tag="oc")
                        nc.tensor.matmul(
                            po[:, :slen],
                            sel_2[:, 0:128],
                            r32(rrow2[:, soff : soff + slen]),
                        )
                        nc.vector.tensor_mul(
                            q2st[:, soff : soff + slen],
                            ts2[:, soff : soff + slen],
                            po[:, :slen],
                        )
                    # align k2 to partitions 0:64 so score matmuls share a
                    # base partition with the q2 moving operand
                    nc.sync.dma_start(k2q[:, :], q2st[64:128, :])

                def ff_pair(i, fb):
                    foff, flen = FB_BLOCKS[fb]
                    of, og = 3 + i, 6 + i
                    # gate matmuls -> silu evac frees the tile -> ffx matmuls
                    # reuse it (keeps psFF at one PSUM bank)
                    pgt = psFF.tile([128, 512], F32, name="pf", tag="pf")
                    pg = pgt[:, 0:QF]
                    for p in range(3):
                        nc.tensor.matmul(
                            pg,
                            wf8s[:, 2 * p : 2 * p + 2,
                                 128 * og : 128 * (og + 1)],
                            xraw[:, 2 * p : 2 * p + 2, foff : foff + flen],
                            start=(p == 0), stop=(p == 2), perf_mode=DR,
                        )
                    gs = pgs.tile([128, QF], BF16, name="gs", tag="gs")
                    nc.scalar.activation(gs[:], pg, AF.Silu)
                    pft = psFF.tile([128, 512], F32, name="pf", tag="pf")
                    pf = pft[:, 0:QF]
                    for p in range(3):
                        nc.tensor.matmul(
                            pf,
                            wf8s[:, 2 * p : 2 * p + 2,
                                 128 * of : 128 * (of + 1)],
                            xraw[:, 2 * p : 2 * p + 2, foff : foff + flen],
                            start=(p == 0), stop=(p == 2), perf_mode=DR,
                        )
                    nc.vector.tensor_mul(
                        dff[:, i, foff : foff + flen], gs[:], pf
                    )

                def d_group(o, fb, pool=None):
                    foff, flen = FB_BLOCKS[fb]
                    acct = (pool or psDG).tile(
                        [128, 512], F32, name="oc", tag="oc"
                    )
                    acc = acct[:, 0:QF]
                    eng = (nc.vector, nc.scalar)[(o + fb) % 2]
                    nc.tensor.matmul(
                        acc, wfb[:, 0:2, 128 * o : 128 * (o + 1)],
                        dff[:, 0:2, foff : foff + flen],
                        start=True, stop=False, perf_mode=DR,
                    )
                    nc.tensor.matmul(
                        acc, wfb[:, 2:4, 128 * o : 128 * (o + 1)],
                        dff[:, 2:4, foff : foff + flen],
                        start=False, stop=False, perf_mode=DR,
                    )
                    nc.tensor.matmul(
                        acc, wa4[:, 0:2, 128 * o : 128 * (o + 1)],
                        dact[:, 0:2, foff : foff + flen],
                        start=False, stop=False, perf_mode=DR,
                    )
                    nc.tensor.matmul(
                        acc, wa4[:, 2:4, 128 * o : 128 * (o + 1)],
                        dact[:, 2:4, foff : foff + flen],
                        start=False, stop=True, perf_mode=DR,
                    )
                    if eng is nc.scalar:
                        nc.scalar.activation(
                            obr[:, o, foff : foff + flen], acc, AF.Copy
                        )
                    else:
                        eng.tensor_copy(obr[:, o, foff : foff + flen], acc)
                    if fb == 2:
                        nc.sync.dma_start(
                            outT[128 * o : 128 * (o + 1), 0 : 3 * QF],
                            obr[:, o, 0 : 3 * QF],
                        )
                    elif fb == 3:
                        nc.sync.dma_start(
                            outT[128 * o : 128 * (o + 1), 3 * QF : S],
                            obr[:, o, 3 * QF : S],
                        )

                # slot processing order: h=2 of quarter 0 deferred to 4th so
                # the qk2 rope/stats pipeline can ride works of slots 1-2
                SLOT_SEQ = [(0, 0), (0, 1), (1, 0), (0, 2), (1, 1), (1, 2),
                            (2, 0), (2, 1), (2, 2), (3, 0), (3, 1), (3, 2)]

                def vpw(js, vi):
                    def f():
                        for i, j in enumerate(js):
                            vproj(j, vi + i, psDG)
                    return f

                def ffw(i, fb):
                    return lambda: ff_pair(i, fb)

                def dgw(*ofs):
                    def f():
                        for o, fb in ofs:
                            d_group(o, fb)
                    return f

                # per-slot extra work, emitted interleaved with attention;
                # slot 0 emits the remaining v tiles just-in-time for its own
                # A*V consumption order (12,13),(2,3),(0,1),(6,7),(4,5),...
                slot_work = {
                    (0, 0): [vpw((2, 3, 0, 1), 2), vpw((6, 7, 4, 5), 6),
                             vpw((10, 11, 8, 9), 10)],
                    (0, 1): [qk2_stats, ffw(0, 0), ffw(1, 0)],
                    (1, 0): [rope_qk2, ffw(2, 0), ffw(0, 1)],
                    (0, 2): [ffw(1, 1), ffw(2, 1)],
                    (1, 1): [ffw(0, 2), dgw((0, 0)), dgw((1, 0))],
                    (1, 2): [ffw(1, 2), dgw((2, 0)), dgw((3, 0))],
                    (2, 0): [ffw(2, 2), dgw((4, 0)), dgw((5, 0))],
                    (2, 1): [ffw(0, 3), dgw((0, 1)), dgw((1, 1))],
                    (2, 2): [ffw(1, 3), dgw((2, 1)), dgw((3, 1))],
                    (3, 0): [ffw(2, 3), dgw((4, 1)), dgw((5, 1))],
                    (3, 1): [dgw((0, 2), (1, 2)), dgw((2, 2)), dgw((3, 2))],
                    (3, 2): [dgw((4, 2)), dgw((5, 2))],
                }

                pb13d = [
                    pm.tile([128, 2, QF], F8, name=f"pb13_{i}", tag=f"pb13_{i}")
                    for i in range(2)
                ]
                for i in range(2):
                    nc.gpsimd.memset(pb13d[i][64:128, 1, :], 0.0)

                def emit_av(av, pos, p, pb, h):
                    j0 = 2 * p
                    nc.tensor.matmul(
                        av[:, :],
                        vx8[:, j0 : j0 + 2, 65 * h : 65 * h + 65],
                        pb[:, :, :],
                        start=(pos == 0), stop=(pos == 6),
                        perf_mode=DR,
                    )

                for sidx, (qf, h) in enumerate(SLOT_SEQ):
                        qoff = QF * qf
                        works = list(slot_work.get((qf, h), ()))
                        av = psAV.tile([65, QF], F32, name="av", tag="av")
                        pend = []
                        for pos, p in enumerate(PAIR_ORDER):
                            j0, j1 = 2 * p, 2 * p + 1
                            pb = pb13d[sidx % 2] if p == 6 else pbf.tile(
                                [128, 2, QF], F8, name="pbf", tag="pbf"
                            )
                            if h == 2:
                                kt_src, qt_src = k2q, q2st
                                kr0, qr0 = 0, 0
                            else:
                                kt_src, qt_src = kst, qst
                                kr0 = qr0 = 64 * h
                            for jj, j in enumerate((j0, j1)):
                                toff, tlen = T_TILES[j]
                                sc = psSC.tile([128, 512], F32,
                                               name="sc", tag="sc")
                                nc.tensor.matmul(
                                    sc[:tlen, 0:QF],
                                    kt_src[kr0 : kr0 + 64, toff : toff + tlen],
                                    qt_src[qr0 : qr0 + 64, qoff : qoff + QF],
                                )
                                if (2 * pos + jj + sidx) % 2 == 0:
                                    nc.scalar.activation(
                                        pb[:tlen, jj, :], sc[:tlen, 0:QF],
                                        AF.Exp, bias=cm8[:tlen, :],
                                    )
                                else:
                                    nc.vector.tensor_scalar(
                                        pb[:tlen, jj, :].bitcast(U8),
                                        sc[:tlen, 0:QF],
                                        ACOEF8, BCONST8, ALU.mult, ALU.add,
                                    )
                            # lag A*V by two pairs so the PE queue never
                            # waits on a just-issued exp
                            pend.append((pos, p, pb))
                            if len(pend) > 2:
                                emit_av(av, *pend.pop(0), h)
                            if pos in WORK_AT and works:
                                works.pop(0)()
                        for ent in pend:
                            emit_av(av, *ent, h)
                        segs = psg.tile([1, QF], F32, name="segs", tag="segs")
                        with nc.allow_low_precision(
                            reason="f32r denominators feed a broadcast matmul"
                        ):
                            nc.vector.reciprocal(r32(segs[:]), av[64:65, :])
                        pobt = psDG.tile([128, 512], F32, name="pob", tag="oc")
                        pob = pobt[0:64, 0:QF]
                        nc.tensor.matmul(pob, e164, r32(segs[:]))
                        nc.vector.tensor_mul(
                            dact[:, h, qoff : qoff + QF], av[0:64, :], pob
                        )
                        for work in works:
                            work()

            # tail: last-quarter output groups get a fresh deep PSUM pool
            with tc.tile_pool(name="psT", bufs=3, space="PSUM") as psT:
                for o in range(6):
                    d_group(o, 3, psT)

    _split_excess_waits(nc)
    return nc


# ---------------------------------------------------------------------------
# host-side preparation
# ---------------------------------------------------------------------------


def _axial_freqs():
    base = np.linspace(1.0, MAX_FREQ / 2, 8) * math.pi

    def ax(n):
        pos = np.linspace(-1.0, 1.0, n)
        return np.repeat(pos[:, None] * base[None, :], 2, axis=-1)

    fH = np.broadcast_to(ax(H)[:, None, None, :], (H, W, D, 16))
    fW = np.broadcast_to(ax(W)[None, :, None, :], (H, W, D, 16))
    fD = np.broadcast_to(ax(D)[None, None, :, :], (H, W, D, 16))
    return np.concatenate((fH, fW, fD), axis=-1).reshape(S, ROT)


def _chunked(mat):
    """[768, C] -> [128, 6, C] (chunk-major rows to partition-major)."""
    C = mat.shape[1]
    return np.ascontiguousarray(mat.reshape(6, 128, C).transpose(1, 0, 2))


def _prep_core_inputs(x, norm1_w, w_fused, b_fused, q_gamma, q_beta, k_gamma,
                      k_beta, w_attn, w_ff, b_ff):
    """Returns list of 8 in_maps (core = b*4 + r)."""
    f64 = np.float64
    F8NP = mybir.dt.np(F8)
    BF16NP = mybir.dt.np(BF16)
    w_fused = np.asarray(w_fused, f64)
    q_gamma = np.asarray(q_gamma, f64)
    k_gamma = np.asarray(k_gamma, f64)

    if np.any(np.asarray(b_fused)) or np.any(np.asarray(b_ff)):
        raise NotImplementedError("nonzero biases not supported by this kernel")
    if np.any(np.asarray(q_beta)) or np.any(np.asarray(k_beta)):
        raise NotImplementedError("nonzero q/k beta not supported by this kernel")
    if np.any(q_gamma == 0) or np.any(k_gamma == 0):
        raise NotImplementedError("zero gamma not supported by this kernel")

    M = np.eye(HD) - np.ones((HD, HD)) / HD
    Aq = np.diag(q_gamma) @ M
    Ak = np.diag(k_gamma) @ M
    R = np.zeros((HD, HD))
    for i in range(ROT // 2):
        R[2 * i, 2 * i + 1] = -1.0
        R[2 * i + 1, 2 * i] = 1.0
    R2 = np.zeros((128, 128))
    R2[0:64, 0:64] = R
    R2[64:128, 64:128] = R

    freqs = _axial_freqs()
    cos64 = np.ones((HD, S))
    sin64 = np.zeros((HD, S))
    cos64[:ROT, :] = np.cos(freqs).T
    sin64[:ROT, :] = np.sin(freqs).T
    cosT = np.vstack([cos64, cos64])
    sinT = np.vstack([sin64, sin64])

    packR = np.zeros((128, PR_COLS))
    packR[:, PR_RR[0] : PR_RR[1]] = R2.T
    packR[:, PR_COS[0] : PR_COS[1]] = cosT
    packR[:, PR_SIN[0] : PR_SIN[1]] = sinT
    packR = packR.astype(BF16NP)

    wq_full = w_fused[MLP : MLP + HID]
    wk_full = w_fused[MLP + HID : MLP + 2 * HID]
    wv_full = w_fused[MLP + 2 * HID :]
    ffx_full = w_fused[: MLP // 2]
    gate_full = w_fused[MLP // 2 : MLP]

    nw = np.asarray(norm1_w, np.float32).reshape(6, 128).T
    iq = 1.0 / (HD * q_gamma**2)
    ik = 1.0 / k_gamma**2
    wq01 = np.zeros((128, 2))
    wq01[0:64, 0] = iq
    wq01[64:128, 1] = iq
    wk01 = np.zeros((128, 2))
    wk01[0:64, 0] = ik
    wk01[64:128, 1] = ik
    wqk2 = np.zeros((128, 2))
    wqk2[0:64, 0] = iq
    wqk2[64:128, 1] = ik

    packS = np.zeros((128, PS_COLS), np.float32)
    packS[:, PS_NW[0] : PS_NW[1]] = nw
    packS[0, PS_SELQ[0] : PS_SELQ[0] + 64] = 1.0
    packS[1, PS_SELQ[0] + 64 : PS_SELQ[0] + 128] = 1.0
    packS[2, PS_SELK[0] : PS_SELK[0] + 64] = 1.0
    packS[3, PS_SELK[0] + 64 : PS_SELK[0] + 128] = 1.0
    packS[0, PS_SEL2[0] : PS_SEL2[0] + 64] = 1.0
    packS[1, PS_SEL2[0] + 64 : PS_SEL2[0] + 128] = 1.0
    packS[0, PS_E164[0] : PS_E164[1]] = 1.0
    packS[0:4, PS_EPS4[0]] = [EPS_LN, EPS_LN, 64 * EPS_LN, 64 * EPS_LN]
    packS[0:2, PS_EPS2[0]] = [EPS_LN, 64 * EPS_LN]

    packB = np.zeros((128, PB_COLS))
    packB[:, PB_WQ4[0] : PB_WQ4[0] + 2] = wq01
    packB[:, PB_WK4[0] + 2 : PB_WK4[0] + 4] = wk01
    packB[:, PB_WQK2[0] : PB_WQK2[1]] = wqk2
    packB = packB.astype(BF16NP)

    w_attn = np.asarray(w_attn, f64)
    w_ff = np.asarray(w_ff, f64)
    in_maps = []
    for core in range(N_CORES):
        b, r = divmod(core, TP)
        hs = [HPC * r + i for i in range(HPC)]
        q3 = [Aq @ wq_full[HD * h : HD * (h + 1)] for h in hs]
        k3 = [Ak @ wk_full[HD * h : HD * (h + 1)] for h in hs]
        ffx = ffx_full[FFPC * r : FFPC * (r + 1)]
        gate = gate_full[FFPC * r : FFPC * (r + 1)]
        wf_mat = np.vstack(
            [q3[0], q3[1], k3[0], k3[1], q3[2], k3[2], ffx, gate]
        ).T  # [HID, NF]
        wv_mat = np.zeros((VP, HID))
        for i, h in enumerate(hs):
            wv_mat[65 * i : 65 * i + HD] = wv_full[HD * h : HD * (h + 1)]
        wa4_np = np.zeros((64, 4, HID))
        wa4_np[:, 0, :] = w_attn[:, HD * hs[0] : HD * hs[0] + HD].T
        wa4_np[:, 1, :] = w_attn[:, HD * hs[1] : HD * hs[1] + HD].T
        wa4_np[:, 2, :] = w_attn[:, HD * hs[2] : HD * hs[2] + HD].T
        wffr = w_ff[:, FFPC * r : FFPC * (r + 1)]
        wfb_np = np.zeros((128, 4, HID))
        wfb_np[:, 0, :] = wffr[:, 0:128].T
        wfb_np[:, 1, :] = wffr[:, 128:256].T
        wfb_np[:, 2, :] = wffr[:, 256:384].T
        in_maps.append(
            {
                "xT": _chunked(
                    np.asarray(x[b], np.float32).reshape(HID, S)
                ).astype(F8NP),
                "wfT": _chunked(wf_mat).astype(BF16NP),
                "wvT": _chunked(wv_mat.T).astype(BF16NP),
                "wa4T": wa4_np.astype(F8NP),
                "wfbT": wfb_np.astype(F8NP),
                "packST": packS,
                "packBT": packB,
                "packRT": packR,
            }
        )
    return in_maps


_NC_CACHE = {}


def get_program():
    if "nc" not in _NC_CACHE:
        _NC_CACHE["nc"] = build_program()
    return _NC_CACHE["nc"]


def kernel(**inputs) -> np.ndarray:
    nc = get_program()
    in_maps = _prep_core_inputs(**inputs)
    res = bass_utils.run_bass_kernel_spmd(nc, in_maps, core_ids=list(range(N_CORES)))
    out = np.zeros((B, HID, H, W, D), np.float32)
    for core in range(N_CORES):
        b = core // TP
        out[b] += res.results[core]["outT"].astype(np.float32).reshape(
            HID, H, W, D
        )
    out += np.asarray(inputs["x"], np.float32)
    return out


# revision 51
# speedup vs baseline: 1.0844x; 1.0047x over previous
"""Trainium2 Bass kernel for nn_FullAttention_17789754540074.

Self-contained: takes the FULL inputs of reference.setup_inputs(), returns the
FULL output. Internally shards across 8 NeuronCores as 2-way data parallel
(batch) x 4-way tensor parallel (3 heads + 384 FF pairs per rank), runs one
SPMD Bass/Tile program via run_bass_kernel_spmd, and sums the 4 partial
outputs per batch on the host, adding the residual there too (the unshard
step for partial-sum TP sharding).

v2 structure (vs the 161us baseline):
  - x ships fp8; the RMS-norm scale is folded into the fused/v WEIGHTS
    (1152+208 cols per chunk on the scale pass instead of 1728), so the
    QKV/ff/v matmuls consume raw fp8 x directly
  - inputs arrive in 15 DMAs (constant packs by dtype + priority) instead of
    47; outputs leave in 12 per-o row DMAs instead of 24 quarter DMAs
  - softmax: all score pairs land in [128,2,512] PSUM tiles; ACT pairs use a
    single merged exp per pair; DVE pairs use a uint8 Schraudolph writing
    fp8e4 probs directly (saturates at 0 below, bits<=126 above by XBIAS
    choice), so ALL A*V matmuls run fp8 DoubleRow and the bf16 v copy dies
  - per-slot normalize fused: dact = av[0:64] * pob with both operands in
    PSUM (no separate att evacuation)
  - rope cos/sin muls run bf16 2x on DVE instead of Pool
"""

import math

import numpy as np

import concourse.bass as bass
import concourse.mybir as mybir
import concourse.tile as tile
from concourse import bass_utils
from concourse.vector_clock import ScopedClock

F32 = mybir.dt.float32
F32R = mybir.dt.float32r
F8 = mybir.dt.float8e4
BF16 = mybir.dt.bfloat16
U8 = mybir.dt.uint8
AF = mybir.ActivationFunctionType
ALU = mybir.AluOpType
DR = mybir.MatmulPerfMode.DoubleRow

HID, HEADS, HD, MLP = 768, 12, 64, 3072
B, H, W, D = 2, 12, 12, 12
S = H * W * D  # 1728
ROT = 48
MAX_FREQ = 256.0
EPS_GN, EPS_LN = 1e-6, 1e-5

N_CORES = 8
TP = 4
HPC = 3  # heads per core
FFPC = 384  # ff pairs per core
NF = 9 * 128  # fused rows: q01 | k01 | q2k2 | ffx*3 | gate*3
VP = 208  # v proj cols: [v0|1|v1|1|v2|1] = 195 used + pad

S_BLOCKS = [(0, 512), (512, 512), (1024, 448), (1472, 256)]
T_TILES = [(128 * j, 128) for j in range(13)] + [(1664, 64)]
QF = S // 4  # 432 queries per attention slot
FB_BLOCKS = [(QF * q, QF) for q in range(4)]
TR_BLOCKS = [(0, 512), (512, 512), (1024, 512), (1536, 192)]

# Softmax exp bias: keeps Schraudolph uint8 bits <= ~120 (below the 0x7F NaN
# encoding) for scores up to ~8, while bits<0 saturate to 0 (prob 0), roughly
# matching the fp8 subnormal flush of the ACT exp path.
XBIAS = -2.5
# fp8e4m3 Schraudolph: u8 = sc*ACOEF8 + BCONST8; bits(u8) ~ e4m3(exp(sc+XBIAS))
ACOEF8 = float(8.0 / math.log(2.0))
BCONST8 = float(8.0 * (XBIAS / math.log(2.0) + 7.0) - 0.490)
# which key-tile pairs per slot run on DVE (rest on ACT)
DVE_PAIRS = (1, 3, 5)
# pair emission order within a slot: alternate ACT/DVE consumers, start with
# the odd p=6 pair so the slot tail ends on fast pairs
PAIR_ORDER = (6, 1, 0, 3, 2, 5, 4)
# emit interleaved work at these POSITIONS in PAIR_ORDER (the DVE pairs, so
# the works' ACT/PE load lands while ACT is otherwise idle)
WORK_AT = (1, 3, 5)


# constant-pack column layout (see _prep_core_inputs)
PS_NW = (0, 6)
PS_SELQ = (6, 134)
PS_SELK = (134, 262)
PS_SEL2 = (262, 390)
PS_E164 = (390, 454)
PS_EPS4 = (454, 455)  # rows: EPS, EPS, 64*EPS, 64*EPS
PS_EPS2 = (455, 456)  # rows: EPS, 64*EPS
PS_COLS = 456
PB_WQ4 = (0, 4)  # cols (iq_q0, iq_q1, 0, 0)
PB_WK4 = (4, 8)  # cols (0, 0, ik_k0, ik_k1)
PB_WQK2 = (8, 10)
PB_COLS = 10
PR_RR = (0, 128)
PR_COS = (128, 128 + S)
PR_SIN = (128 + S, 128 + 2 * S)
PR_COLS = 128 + 2 * S


class TileContextSplitDrain(tile.TileContext):
    """TileContext whose kernel-tail drain splits its semaphore waits across
    single-wait sync NOPs — the walrus build here rejects >2 sync waits on one
    SP CTRL instruction ("Too many sync wait commands")."""

    def _drain_and_barrier(self, tick_clock, wait_clock):
        probe = self.nc.sync.nop(nofuse=True)
        wait_clock.add_sem_waits(
            probe.ins, ScopedClock({None: tick_clock.global_clock})
        )
        si = probe.ins.sync_info
        waits = list(si.on_wait) if si is not None else []
        if si is not None:
            si.on_wait = waits[:1]
        for w in waits[1:]:
            n = self.nc.sync.nop(nofuse=True)
            nsi = n.ins.sync_info
            if nsi is None:
                n.ins.sync_info = mybir.SyncInfo(on_wait=[w], on_update=[])
            else:
                nsi.on_wait.append(w)
        self.nc.sync.drain()
        self.nc.all_engine_barrier()
        popped = self.nc._tile_sem_poison_stack.pop()
        assert popped is self._sem_poison
        self.nc.clear_and_free_semaphores(list(self.sems.allocated().values()))
        self.nc.all_engine_barrier()


def r32(ap):
    return ap.bitcast(F32R)


def _split_excess_waits(nc, maxw=1):
    """walrus in this container caps sync waits per instruction; move extras
    onto preceding same-engine NOPs (waits execute in program order)."""
    nid = 0
    for bb in nc.m.functions[0].blocks:
        insts = bb.instructions
        i = 0
        while i < len(insts):
            inst = insts[i]
            si = inst.sync_info
            nw = len(si.on_wait) if si is not None and si.on_wait else 0
            if nw > maxw:
                waits = list(si.on_wait)
                si.on_wait = waits[-maxw:]
                extra = waits[:-maxw]
                pos = i
                for k in range(0, len(extra), maxw):
                    nop = mybir.InstNoOp(
                        name=f"I-waitsplit-{nid}", ins=[], outs=[]
                    )
                    nop.engine = inst.engine
                    nop.sync_info = mybir.SyncInfo(
                        on_wait=extra[k : k + maxw], on_update=[]
                    )
                    insts.insert(pos, nop)
                    nc.register_instruction(nop)
                    pos += 1
                    i += 1
                    nid += 1
            i += 1


def build_program():
    nc = bass.Bass(trn_type="TRN2")

    xT = nc.dram_tensor("xT", [128, 6, S], F8, kind="ExternalInput")
    wfT = nc.dram_tensor("wfT", [128, 6, NF], BF16, kind="ExternalInput")
    wvT = nc.dram_tensor("wvT", [128, 6, VP], BF16, kind="ExternalInput")
    wa4T = nc.dram_tensor("wa4T", [64, 4, HID], F8, kind="ExternalInput")
    wfbT = nc.dram_tensor("wfbT", [128, 4, HID], F8, kind="ExternalInput")
    packST = nc.dram_tensor("packST", [128, PS_COLS], F32, kind="ExternalInput")
    packBT = nc.dram_tensor("packBT", [128, PB_COLS], BF16, kind="ExternalInput")
    packRT = nc.dram_tensor("packRT", [128, PR_COLS], BF16, kind="ExternalInput")
    outT = nc.dram_tensor("outT", [HID, S], BF16, kind="ExternalOutput")

    with TileContextSplitDrain(nc) as tc:
        with tc.tile_pool(name="main", bufs=1) as pm:
            # ---- long-lived SBUF tiles --------------------------------------
            xraw = pm.tile([128, 6, S], F8, name="xraw", tag="xraw")
            wf16 = pm.tile([128, 6, NF], BF16, name="wf16", tag="wf16")
            wv16 = pm.tile([128, 6, VP], BF16, name="wv16", tag="wv16")
            wf8s = pm.tile([128, 6, NF], F8, name="wf8s", tag="wf8s")
            wv8s = pm.tile([128, 6, VP], F8, name="wv8s", tag="wv8s")
            wa4 = pm.tile([64, 4, HID], F8, name="wa4", tag="wa4")
            wfb = pm.tile([128, 4, HID], F8, name="wfb", tag="wfb")
            packS = pm.tile([128, PS_COLS], F32, name="packS", tag="packS")
            packB = pm.tile([128, PB_COLS], BF16, name="packB", tag="packB")
            packR = pm.tile([128, PR_COLS], BF16, name="packR", tag="packR")
            qab = pm.tile([128, S], BF16, name="qab", tag="qab")
            kab = pm.tile([128, S], BF16, name="kab", tag="kab")
            qk2 = pm.tile([128, S], BF16, name="qk2", tag="qk2")
            qst = pm.tile([128, S], F8, name="qst", tag="qst")
            kst = pm.tile([128, S], F8, name="kst", tag="kst")
            q2st = pm.tile([128, S], F8, name="q2st", tag="q2st")
            k2q = pm.tile([64, S], F8, name="k2q", tag="k2q")
            vx8 = pm.tile([128, 14, VP], F8, name="vx8", tag="vx8")
            dact = pm.tile([HD, 4, S], F8, name="dact", tag="dact")
            dff = pm.tile([128, 4, S], F8, name="dff", tag="dff")
            sqq = pm.tile([128, S], BF16, name="sqq", tag="sqq")
            sqk = pm.tile([128, S], BF16, name="sqk", tag="sqk")
            sq2 = pm.tile([128, S], BF16, name="sq2", tag="sq2")
            sqju = pm.tile([128, 2, S], F8, name="sqju", tag="sqju")
            sqh = pm.tile([128, 2, 512], BF16, name="sqh", tag="sqh")
            ssb = pm.tile([128, 6], F32, name="ssb", tag="ssb")
            lnr = pm.tile([4, S], F32, name="lnr", tag="lnr")
            lnr2 = pm.tile([2, S], F32, name="lnr2", tag="lnr2")
            rrow = pm.tile([4, S], F32, name="rrow", tag="rrow")
            rrow2 = pm.tile([2, S], F32, name="rrow2", tag="rrow2")
            ss12 = pm.tile([128, 6], F32, name="ss12", tag="ss12")
            rmsc = pm.tile([128, 6], F32, name="rmsc", tag="rmsc")
            scl6 = pm.tile([128, 6], F32, name="scl6", tag="scl6")
            cgn = pm.tile([128, 1], F32, name="cgn", tag="cgn")
            cm8 = pm.tile([128, 1], F32, name="cm8", tag="cm8")
            tsq = pm.tile([128, S], BF16, name="tsq", tag="tsq")
            tcq = pm.tile([128, S], BF16, name="tcq", tag="tcq")
            tsk = pm.tile([128, S], BF16, name="tsk", tag="tsk")
            tck = pm.tile([128, S], BF16, name="tck", tag="tck")
            obr = pm.tile([128, 6, S], BF16, name="obr", tag="obr")

            # constant-pack slices
            nw = packS[:, PS_NW[0] : PS_NW[1]]
            sel_q = r32(packS[0:4, PS_SELQ[0] : PS_SELQ[1]])
            sel_k = r32(packS[0:4, PS_SELK[0] : PS_SELK[1]])
            sel_2 = r32(packS[0:2, PS_SEL2[0] : PS_SEL2[1]])
            e164 = r32(packS[0:1, PS_E164[0] : PS_E164[1]])
            wq4 = packB[:, PB_WQ4[0] : PB_WQ4[1]]
            wk4 = packB[:, PB_WK4[0] : PB_WK4[1]]
            wqk2 = packB[:, PB_WQK2[0] : PB_WQK2[1]]
            eps4 = packS[0:4, PS_EPS4[0] : PS_EPS4[1]]
            eps2 = packS[0:2, PS_EPS2[0] : PS_EPS2[1]]
            rrm = packR[:, PR_RR[0] : PR_RR[1]]
            cosb = packR[:, PR_COS[0] : PR_COS[1]]
            sinb = packR[:, PR_SIN[0] : PR_SIN[1]]

            # ---- input DMAs (priority order) --------------------------------
            nc.sync.dma_start(packS[:], packST[:])
            for c in range(6):
                nc.sync.dma_start(xraw[:, c, :], xT[:, c, :])
            nc.sync.dma_start(packB[:], packBT[:])
            for p in range(3):
                nc.sync.dma_start(
                    wf16[:, 2 * p : 2 * p + 2, :], wfT[:, 2 * p : 2 * p + 2, :]
                )
            nc.sync.dma_start(packR[:], packRT[:])
            nc.sync.dma_start(wv16[:], wvT[:])
            nc.sync.dma_start(wa4[:], wa4T[:])
            nc.sync.dma_start(wfb[:], wfbT[:])

            # ---- constants / zero pads --------------------------------------
            nc.vector.memset(cgn[:], EPS_GN)
            nc.vector.memset(cm8[:], XBIAS)

            nc.gpsimd.memset(dact[:, 3, :], 0.0)
            nc.gpsimd.memset(dff[:, 3, :], 0.0)


            # ---- phase A: rms norm stats + weight scaling + fused qkv -------
            with (
                tc.tile_pool(name="psA", bufs=3, space="PSUM") as psA,
                tc.tile_pool(name="psRP", bufs=3, space="PSUM") as psRP,
            ):
                SPL = 1216
                for c in range(6):
                    nc.scalar.activation(
                        sqju[:, c % 2, 0:SPL], xraw[:, c, 0:SPL], AF.Square,
                        accum_out=ss12[:, c : c + 1],
                    )
                    nc.vector.tensor_mul(
                        sqh[:, c % 2, :], xraw[:, c, SPL:S], xraw[:, c, SPL:S]
                    )
                    nc.vector.tensor_reduce(
                        ssb[:, c : c + 1], sqh[:, c % 2, :],
                        mybir.AxisListType.X, ALU.add,
                    )
                    nc.vector.tensor_add(
                        ss12[:, c : c + 1], ss12[:, c : c + 1],
                        ssb[:, c : c + 1],
                    )
                    nc.scalar.activation(
                        rmsc[:, c : c + 1], ss12[:, c : c + 1], AF.Sqrt,
                        bias=cgn[:], scale=1.0 / S,
                    )
                    nc.vector.reciprocal(
                        scl6[:, c : c + 1], rmsc[:, c : c + 1]
                    )
                    nc.vector.tensor_mul(
                        scl6[:, c : c + 1], scl6[:, c : c + 1], nw[:, c : c + 1]
                    )
                    # scale weights (not x): wf8s = fp8(wf16 * scl), ditto wv
                    weng = (nc.gpsimd, nc.gpsimd, nc.scalar,
                            nc.vector, nc.scalar, nc.vector)[c]
                    if weng is nc.scalar:
                        nc.scalar.activation(
                            wf8s[:, c, :], wf16[:, c, :], AF.Copy,
                            scale=scl6[:, c : c + 1],
                        )
                    else:
                        weng.tensor_scalar(
                            wf8s[:, c, :], wf16[:, c, :],
                            scl6[:, c : c + 1], None, ALU.mult,
                        )
                    nc.gpsimd.tensor_scalar(
                        wv8s[:, c, :], wv16[:, c, :],
                        scl6[:, c : c + 1], None, ALU.mult,
                    )

                # qkv q01/k01 blocks first (o=0,1), evacs spread over engines
                qk_dst = [qab, kab, qk2]

                def qkv_block(o, sb):
                    soff, slen = S_BLOCKS[sb]
                    pt = psA.tile([128, 512], F32, name="mm", tag="mm")
                    acc = pt[:, :slen]
                    for p in range(3):
                        nc.tensor.matmul(
                            acc,
                            wf8s[:, 2 * p : 2 * p + 2,
                                 128 * o : 128 * (o + 1)],
                            xraw[:, 2 * p : 2 * p + 2, soff : soff + slen],
                            start=(p == 0), stop=(p == 2), perf_mode=DR,
                        )
                    eng = (nc.vector, nc.scalar)[(o + sb) % 2]
                    if eng is nc.scalar:
                        nc.scalar.activation(
                            qk_dst[o][:, soff : soff + slen], acc, AF.Copy
                        )
                    else:
                        eng.tensor_copy(qk_dst[o][:, soff : soff + slen], acc)

                for o in range(2):
                    for sb in range(4):
                        qkv_block(o, sb)

                # ---- rope sin/cos parts (no stats dependency) ---------------
                # per-block: sin-mul on DVE (reads PSUM), cos-mul on Pool,
                # add on DVE (bf16 2x) — q and k chains overlap; the squares
                # for the token stats ride per-block on ACT
                for src, ts_t, tc_t, sq_t in (
                    (qab, tsq, tcq, sqq), (kab, tsk, tck, sqk)
                ):
                    for soff, slen in S_BLOCKS:
                        rot = psRP.tile([128, 512], F32, name="rot", tag="rp")
                        nc.tensor.matmul(
                            rot[:, :slen], rrm, src[:, soff : soff + slen]
                        )
                        nc.gpsimd.tensor_mul(
                            tc_t[:, soff : soff + slen],
                            src[:, soff : soff + slen],
                            cosb[:, soff : soff + slen],
                        )
                        nc.scalar.activation(
                            sq_t[:, soff : soff + slen],
                            src[:, soff : soff + slen], AF.Square,
                        )
                        nc.vector.tensor_mul(
                            ts_t[:, soff : soff + slen],
                            rot[:, :slen],
                            sinb[:, soff : soff + slen],
                        )
                        nc.vector.tensor_add(
                            ts_t[:, soff : soff + slen],
                            ts_t[:, soff : soff + slen],
                            tc_t[:, soff : soff + slen],
                        )

                # ---- q01/k01 rstd stats, directly in row layout -------------
                # per-token variance sums land as [4, block] rows via
                # moving=squares matmuls; rstd = exp(-0.5 * ln(sum + eps))
                for soff, slen in S_BLOCKS:
                    pr = psA.tile([4, 512], F32, name="vk", tag="mm")
                    nc.tensor.matmul(
                        pr[:, :slen], wq4, sqq[:, soff : soff + slen],
                        start=True, stop=False,
                    )
                    nc.tensor.matmul(
                        pr[:, :slen], wk4, sqk[:, soff : soff + slen],
                        start=False, stop=True,
                    )
                    nc.scalar.activation(
                        lnr[:, soff : soff + slen], pr[:, :slen],
                        AF.Ln, bias=eps4,
                    )
                    nc.scalar.activation(
                        r32(rrow[0:4, soff : soff + slen]),
                        lnr[:, soff : soff + slen], AF.Exp, scale=-0.5,
                    )

                # ---- apply rstd + quantize ----------------------------------
                # kst blocks in the order the first slots consume them
                # (pair order hits key tiles (12,13),(2,3),(0,1),(6,7),...)
                for sb in (3, 0, 1, 2):
                    soff, slen = S_BLOCKS[sb]
                    po = psRP.tile([128, 512], F32, name="pok", tag="rp")
                    nc.tensor.matmul(
                        po[:, :slen],
                        sel_k[:, 0:128],
                        r32(rrow[0:4, soff : soff + slen]),
                    )
                    nc.vector.tensor_mul(
                        kst[:, soff : soff + slen],
                        tsk[:, soff : soff + slen],
                        po[:, :slen],
                    )
                for soff, slen in S_BLOCKS:
                    po = psRP.tile([128, 512], F32, name="po", tag="rp")
                    nc.tensor.matmul(
                        po[:, :slen],
                        sel_q[:, 0:128],
                        r32(rrow[0:4, soff : soff + slen]),
                    )
                    nc.vector.tensor_mul(
                        qst[:, soff : soff + slen],
                        tsq[:, soff : soff + slen],
                        po[:, :slen],
                    )

                # ---- qk2 fused block (evac + square for the C window) -------
                for sb in range(4):
                    qkv_block(2, sb)
                nc.gpsimd.tensor_mul(sq2[:], qk2[:], qk2[:])

                # ---- v projection: only the p6 pair tiles before phase C ----
                def vproj(j, vi, pool=None):
                    toff, tlen = T_TILES[j]
                    pool = pool or psA
                    tag = "mm" if pool is psA else "oc"
                    pt = pool.tile([128, 512], F32, name="mmv", tag=tag)
                    acc = pt[:tlen, :VP]
                    for p in range(3):
                        nc.tensor.matmul(
                            acc,
                            xraw[:, 2 * p : 2 * p + 2, toff : toff + tlen],
                            wv8s[:, 2 * p : 2 * p + 2, :],
                            start=(p == 0), stop=(p == 2), perf_mode=DR,
                        )
                    eng = (nc.vector, nc.scalar)[vi % 2]
                    if eng is nc.scalar:
                        nc.scalar.activation(
                            vx8[:tlen, j, 0:195], acc[:, 0:195], AF.Copy
                        )
                    else:
                        eng.tensor_copy(vx8[:tlen, j, 0:195], acc[:, 0:195])
                    nc.gpsimd.memset(vx8[:tlen, j, 64:195:65], 1.0)

                nc.gpsimd.memset(vx8[64:128, 13, :], 0.0)
                vproj(12, 0)
                vproj(13, 1)

            # ---- phase C: attention + interleaved ff / output ---------------
            with (
                tc.tile_pool(name="psSC", bufs=4, space="PSUM") as psSC,
                tc.tile_pool(name="psAV", bufs=2, space="PSUM") as psAV,
                tc.tile_pool(name="psFF", bufs=1, space="PSUM") as psFF,
                tc.tile_pool(name="psDG", bufs=1, space="PSUM") as psDG,
                tc.tile_pool(name="pbf", bufs=4) as pbf,
                tc.tile_pool(name="pgs", bufs=2) as pgs,
                tc.tile_pool(name="psg", bufs=2) as psg,
            ):
                def qk2_stats():
                    for soff, slen in S_BLOCKS:
                        pr = psDG.tile([2, 512], F32, name="vk2", tag="oc")
                        nc.tensor.matmul(
                            pr[:, :slen], wqk2, sq2[:, soff : soff + slen]
                        )
                        nc.scalar.activation(
                            lnr2[:, soff : soff + slen], pr[:, :slen],
                            AF.Ln, bias=eps2,
                        )
                        nc.scalar.activation(
                            r32(rrow2[:, soff : soff + slen]),
                            lnr2[:, soff : soff + slen], AF.Exp, scale=-0.5,
                        )

                def rope_qk2():
                    ts2 = pm.tile([128, S], BF16, name="ts2", tag="ts2")
                    tc2 = pm.tile([128, S], BF16, name="tc2", tag="tc2")
                    for soff, slen in S_BLOCKS:
                        rot = psDG.tile([128, 512], F32, name="rot2", tag="oc")
                        nc.tensor.matmul(
                            rot[:, :slen], rrm, qk2[:, soff : soff + slen]
                        )
                        nc.vector.tensor_mul(
                            ts2[:, soff : soff + slen],
                            rot[:, :slen],
                            sinb[:, soff : soff + slen],
                        )
                    nc.vector.tensor_mul(tc2[:], qk2[:], cosb)
                    nc.vector.tensor_add(ts2[:], ts2[:], tc2[:])
                    # rows 0-63 (q2) scaled by rstd_q2; 64-127 (k2) by rstd_k2/8
                    for soff, slen in S_BLOCKS:
                        po = psDG.tile([128, 512], F32, name="po2", tag="oc")
                        nc.tensor.matmul(
                            po[:, :slen],
                            sel_2[:, 0:128],
                            r32(rrow2[:, soff : soff + slen]),
                        )
                        nc.vector.tensor_mul(
                            q2st[:, soff : soff + slen],
                            ts2[:, soff : soff + slen],
                            po[:, :slen],
                        )
                    # align k2 to partitions 0:64 so score matmuls share a
                    # base partition with the q2 moving operand
                    nc.sync.dma_start(k2q[:, :], q2st[64:128, :])

                def ff_pair(i, fb):
                    foff, flen = FB_BLOCKS[fb]
                    of, og = 3 + i, 6 + i
                    # gate matmuls -> silu evac frees the tile -> ffx matmuls
                    # reuse it (keeps psFF at one PSUM bank)
                    pgt = psFF.tile([128, 512], F32, name="pf", tag="pf")
                    pg = pgt[:, 0:QF]
                    for p in range(3):
                        nc.tensor.matmul(
                            pg,
                            wf8s[:, 2 * p : 2 * p + 2,
                                 128 * og : 128 * (og + 1)],
                            xraw[:, 2 * p : 2 * p + 2, foff : foff + flen],
                            start=(p == 0), stop=(p == 2), perf_mode=DR,
                        )
                    gs = pgs.tile([128, QF], BF16, name="gs", tag="gs")
                    nc.scalar.activation(gs[:], pg, AF.Silu)
                    pft = psFF.tile([128, 512], F32, name="pf", tag="pf")
                    pf = pft[:, 0:QF]
                    for p in range(3):
                        nc.tensor.matmul(
                            pf,
                            wf8s[:, 2 * p : 2 * p + 2,
                                 128 * of : 128 * (of + 1)],
                            xraw[:, 2 * p : 2 * p + 2, foff : foff + flen],
                            start=(p == 0), stop=(p == 2), perf_mode=DR,
                        )
                    nc.vector.tensor_mul(
                        dff[:, i, foff : foff + flen], gs[:], pf
                    )

                def d_group(o, fb, pool=None):
                    foff, flen = FB_BLOCKS[fb]
                    acct = (pool or psDG).tile(
                        [128, 512], F32, name="oc", tag="oc"
                    )
                    acc = acct[:, 0:QF]
                    eng = (nc.vector, nc.scalar)[(o + fb) % 2]
                    nc.tensor.matmul(
                        acc, wfb[:, 0:2, 128 * o : 128 * (o + 1)],
                        dff[:, 0:2, foff : foff + flen],
                        start=True, stop=False, perf_mode=DR,
                    )
                    nc.tensor.matmul(
                        acc, wfb[:, 2:4, 128 * o : 128 * (o + 1)],
                        dff[:, 2:4, foff : foff + flen],
                        start=False, stop=False, perf_mode=DR,
                    )
                    nc.tensor.matmul(
                        acc, wa4[:, 0:2, 128 * o : 128 * (o + 1)],
                        dact[:, 0:2, foff : foff + flen],
                        start=False, stop=False, perf_mode=DR,
                    )
                    nc.tensor.matmul(
                        acc, wa4[:, 2:4, 128 * o : 128 * (o + 1)],
                        dact[:, 2:4, foff : foff + flen],
                        start=False, stop=True, perf_mode=DR,
                    )
                    if eng is nc.scalar:
                        nc.scalar.activation(
                            obr[:, o, foff : foff + flen], acc, AF.Copy
                        )
                    else:
                        eng.tensor_copy(obr[:, o, foff : foff + flen], acc)
                    if fb == 2:
                        nc.sync.dma_start(
                            outT[128 * o : 128 * (o + 1), 0 : 3 * QF],
                            obr[:, o, 0 : 3 * QF],
                        )
                    elif fb == 3:
                        nc.sync.dma_start(
                            outT[128 * o : 128 * (o + 1), 3 * QF : S],
                            obr[:, o, 3 * QF : S],
                        )

                # slot processing order: h=2 of quarter 0 deferred to 4th so
                # the qk2 rope/stats pipeline can ride works of slots 1-2
                SLOT_SEQ = [(0, 0), (0, 1), (1, 0), (0, 2), (1, 1), (1, 2),
                            (2, 0), (2, 1), (2, 2), (3, 0), (3, 1), (3, 2)]

                def vpw(js, vi):
                    def f():
                        for i, j in enumerate(js):
                            vproj(j, vi + i, psDG)
                    return f

                def ffw(i, fb):
                    return lambda: ff_pair(i, fb)

                def dgw(*ofs):
                    def f():
                        for o, fb in ofs:
                            d_group(o, fb)
                    return f

                # per-slot extra work, emitted interleaved with attention;
                # slot 0 emits the remaining v tiles just-in-time for its own
                # A*V consumption order (12,13),(2,3),(0,1),(6,7),(4,5),...
                slot_work = {
                    (0, 0): [vpw((2, 3, 0, 1), 2), vpw((6, 7, 4, 5), 6),
                             vpw((10, 11, 8, 9), 10)],
                    (0, 1): [qk2_stats, ffw(0, 0), ffw(1, 0)],
                    (1, 0): [rope_qk2, ffw(2, 0), ffw(0, 1)],
                    (0, 2): [ffw(1, 1), ffw(2, 1)],
                    (1, 1): [ffw(0, 2), dgw((0, 0)), dgw((1, 0))],
                    (1, 2): [ffw(1, 2), dgw((2, 0)), dgw((3, 0))],
                    (2, 0): [ffw(2, 2), dgw((4, 0)), dgw((5, 0))],
                    (2, 1): [ffw(0, 3), dgw((0, 1)), dgw((1, 1))],
                    (2, 2): [ffw(1, 3), dgw((2, 1)), dgw((3, 1))],
                    (3, 0): [ffw(2, 3), dgw((4, 1)), dgw((5, 1))],
                    (3, 1): [dgw((0, 2), (1, 2)), dgw((2, 2)), dgw((3, 2))],
                    (3, 2): [dgw((4, 2)), dgw((5, 2))],
                }

                pb13d = [
                    pm.tile([128, 2, QF], F8, name=f"pb13_{i}", tag=f"pb13_{i}")
                    for i in range(2)
                ]
                for i in range(2):
                    nc.gpsimd.memset(pb13d[i][64:128, 1, :], 0.0)

                def emit_av(av, pos, p, pb, h):
                    j0 = 2 * p
                    nc.tensor.matmul(
                        av[:, :],
                        vx8[:, j0 : j0 + 2, 65 * h : 65 * h + 65],
                        pb[:, :, :],
                        start=(pos == 0), stop=(pos == 6),
                        perf_mode=DR,
                    )

                for sidx, (qf, h) in enumerate(SLOT_SEQ):
                        qoff = QF * qf
                        works = list(slot_work.get((qf, h), ()))
                        av = psAV.tile([65, QF], F32, name="av", tag="av")
                        pend = []
                        for pos, p in enumerate(PAIR_ORDER):
                            j0, j1 = 2 * p, 2 * p + 1
                            pb = pb13d[sidx % 2] if p == 6 else pbf.tile(
                                [128, 2, QF], F8, name="pbf", tag="pbf"
                            )
                            if h == 2:
                                kt_src, qt_src = k2q, q2st
                                kr0, qr0 = 0, 0
                            else:
                                kt_src, qt_src = kst, qst
                                kr0 = qr0 = 64 * h
                            for jj, j in enumerate((j0, j1)):
                                toff, tlen = T_TILES[j]
                                sc = psSC.tile([128, 512], F32,
                                               name="sc", tag="sc")
                                nc.tensor.matmul(
                                    sc[:tlen, 0:QF],
                                    kt_src[kr0 : kr0 + 64, toff : toff + tlen],
                                    qt_src[qr0 : qr0 + 64, qoff : qoff + QF],
                                )
                                if (2 * pos + jj + sidx) % 2 == 0:
                                    nc.scalar.activation(
                                        pb[:tlen, jj, :], sc[:tlen, 0:QF],
                                        AF.Exp, bias=cm8[:tlen, :],
                                    )
                                else:
                                    nc.vector.tensor_scalar(
                                        pb[:tlen, jj, :].bitcast(U8),
                                        sc[:tlen, 0:QF],
                                        ACOEF8, BCONST8, ALU.mult, ALU.add,
                                    )
                            # lag A*V by two pairs so the PE queue never
                            # waits on a just-issued exp
                            pend.append((pos, p, pb))
                            if len(pend) > 2:
                                emit_av(av, *pend.pop(0), h)
                            if pos in WORK_AT and works:
                                works.pop(0)()
                        for ent in pend:
                            emit_av(av, *ent, h)
                        segs = psg.tile([1, QF], F32, name="segs", tag="segs")
                        with nc.allow_low_precision(
                            reason="f32r denominators feed a broadcast matmul"
                        ):
                            nc.vector.reciprocal(r32(segs[:]), av[64:65, :])
                        pobt = psDG.tile([128, 512], F32, name="pob", tag="oc")
                        pob = pobt[0:64, 0:QF]
                        nc.tensor.matmul(pob, e164, r32(segs[:]))
                        nc.vector.tensor_mul(
                            dact[:, h, qoff : qoff + QF], av[0:64, :], pob
                        )
                        for work in works:
                            work()

            # tail: last-quarter output groups get a fresh deep PSUM pool
            with tc.tile_pool(name="psT", bufs=3, space="PSUM") as psT:
                for o in range(6):
                    d_group(o, 3, psT)

    _split_excess_waits(nc)
    return nc


# ---------------------------------------------------------------------------
# host-side preparation
# ---------------------------------------------------------------------------


def _axial_freqs():
    base = np.linspace(1.0, MAX_FREQ / 2, 8) * math.pi

    def ax(n):
        pos = np.linspace(-1.0, 1.0, n)
        return np.repeat(pos[:, None] * base[None, :], 2, axis=-1)

    fH = np.broadcast_to(ax(H)[:, None, None, :], (H, W, D, 16))
    fW = np.broadcast_to(ax(W)[None, :, None, :], (H, W, D, 16))
    fD = np.broadcast_to(ax(D)[None, None, :, :], (H, W, D, 16))
    return np.concatenate((fH, fW, fD), axis=-1).reshape(S, ROT)


def _chunked(mat):
    """[768, C] -> [128, 6, C] (chunk-major rows to partition-major)."""
    C = mat.shape[1]
    return np.ascontiguousarray(mat.reshape(6, 128, C).transpose(1, 0, 2))


def _prep_core_inputs(x, norm1_w, w_fused, b_fused, q_gamma, q_beta, k_gamma,
                      k_beta, w_attn, w_ff, b_ff):
    """Returns list of 8 in_maps (core = b*4 + r)."""
    f64 = np.float64
    F8NP = mybir.dt.np(F8)
    BF16NP = mybir.dt.np(BF16)
    w_fused = np.asarray(w_fused, f64)
    q_gamma = np.asarray(q_gamma, f64)
    k_gamma = np.asarray(k_gamma, f64)

    if np.any(np.asarray(b_fused)) or np.any(np.asarray(b_ff)):
        raise NotImplementedError("nonzero biases not supported by this kernel")
    if np.any(np.asarray(q_beta)) or np.any(np.asarray(k_beta)):
        raise NotImplementedError("nonzero q/k beta not supported by this kernel")
    if np.any(q_gamma == 0) or np.any(k_gamma == 0):
        raise NotImplementedError("zero gamma not supported by this kernel")

    M = np.eye(HD) - np.ones((HD, HD)) / HD
    Aq = np.diag(q_gamma) @ M
    Ak = np.diag(k_gamma) @ M
    R = np.zeros((HD, HD))
    for i in range(ROT // 2):
        R[2 * i, 2 * i + 1] = -1.0
        R[2 * i + 1, 2 * i] = 1.0
    R2 = np.zeros((128, 128))
    R2[0:64, 0:64] = R
    R2[64:128, 64:128] = R

    freqs = _axial_freqs()
    cos64 = np.ones((HD, S))
    sin64 = np.zeros((HD, S))
    cos64[:ROT, :] = np.cos(freqs).T
    sin64[:ROT, :] = np.sin(freqs).T
    cosT = np.vstack([cos64, cos64])
    sinT = np.vstack([sin64, sin64])

    packR = np.zeros((128, PR_COLS))
    packR[:, PR_RR[0] : PR_RR[1]] = R2.T
    packR[:, PR_COS[0] : PR_COS[1]] = cosT
    packR[:, PR_SIN[0] : PR_SIN[1]] = sinT
    packR = packR.astype(BF16NP)

    wq_full = w_fused[MLP : MLP + HID]
    wk_full = w_fused[MLP + HID : MLP + 2 * HID]
    wv_full = w_fused[MLP + 2 * HID :]
    ffx_full = w_fused[: MLP // 2]
    gate_full = w_fused[MLP // 2 : MLP]

    nw = np.asarray(norm1_w, np.float32).reshape(6, 128).T
    iq = 1.0 / (HD * q_gamma**2)
    ik = 1.0 / k_gamma**2
    wq01 = np.zeros((128, 2))
    wq01[0:64, 0] = iq
    wq01[64:128, 1] = iq
    wk01 = np.zeros((128, 2))
    wk01[0:64, 0] = ik
    wk01[64:128, 1] = ik
    wqk2 = np.zeros((128, 2))
    wqk2[0:64, 0] = iq
    wqk2[64:128, 1] = ik

    packS = np.zeros((128, PS_COLS), np.float32)
    packS[:, PS_NW[0] : PS_NW[1]] = nw
    packS[0, PS_SELQ[0] : PS_SELQ[0] + 64] = 1.0
    packS[1, PS_SELQ[0] + 64 : PS_SELQ[0] + 128] = 1.0
    packS[2, PS_SELK[0] : PS_SELK[0] + 64] = 1.0
    packS[3, PS_SELK[0] + 64 : PS_SELK[0] + 128] = 1.0
    packS[0, PS_SEL2[0] : PS_SEL2[0] + 64] = 1.0
    packS[1, PS_SEL2[0] + 64 : PS_SEL2[0] + 128] = 1.0
    packS[0, PS_E164[0] : PS_E164[1]] = 1.0
    packS[0:4, PS_EPS4[0]] = [EPS_LN, EPS_LN, 64 * EPS_LN, 64 * EPS_LN]
    packS[0:2, PS_EPS2[0]] = [EPS_LN, 64 * EPS_LN]

    packB = np.zeros((128, PB_COLS))
    packB[:, PB_WQ4[0] : PB_WQ4[0] + 2] = wq01
    packB[:, PB_WK4[0] + 2 : PB_WK4[0] + 4] = wk01
    packB[:, PB_WQK2[0] : PB_WQK2[1]] = wqk2
    packB = packB.astype(BF16NP)

    w_attn = np.asarray(w_attn, f64)
    w_ff = np.asarray(w_ff, f64)
    in_maps = []
    for core in range(N_CORES):
        b, r = divmod(core, TP)
        hs = [HPC * r + i for i in range(HPC)]
        q3 = [Aq @ wq_full[HD * h : HD * (h + 1)] for h in hs]
        k3 = [Ak @ wk_full[HD * h : HD * (h + 1)] for h in hs]
        ffx = ffx_full[FFPC * r : FFPC * (r + 1)]
        gate = gate_full[FFPC * r : FFPC * (r + 1)]
        wf_mat = np.vstack(
            [q3[0], q3[1], k3[0], k3[1], q3[2], k3[2], ffx, gate]
        ).T  # [HID, NF]
        wv_mat = np.zeros((VP, HID))
        for i, h in enumerate(hs):
            wv_mat[65 * i : 65 * i + HD] = wv_full[HD * h : HD * (h + 1)]
        wa4_np = np.zeros((64, 4, HID))
        wa4_np[:, 0, :] = w_attn[:, HD * hs[0] : HD * hs[0] + HD].T
        wa4_np[:, 1, :] = w_attn[:, HD * hs[1] : HD * hs[1] + HD].T
        wa4_np[:, 2, :] = w_attn[:, HD * hs[2] : HD * hs[2] + HD].T
        wffr = w_ff[:, FFPC * r : FFPC * (r + 1)]
        wfb_np = np.zeros((128, 4, HID))
        wfb_np[:, 0, :] = wffr[:, 0:128].T
        wfb_np[:, 1, :] = wffr[:, 128:256].T
        wfb_np[:, 2, :] = wffr[:, 256:384].T
        in_maps.append(
            {
                "xT": _chunked(
                    np.asarray(x[b], np.float32).reshape(HID, S)
                ).astype(F8NP),
                "wfT": _chunked(wf_mat).astype(BF16NP),
                "wvT": _chunked(wv_mat.T).astype(BF16NP),
                "wa4T": wa4_np.astype(F8NP),
                "wfbT": wfb_np.astype(F8NP),
                "packST": packS,
                "packBT": packB,
                "packRT": packR,
            }
        )
    return in_maps


_NC_CACHE = {}


def get_program():
    if "nc" not in _NC_CACHE:
        _NC_CACHE["nc"] = build_program()
    return _NC_CACHE["nc"]


def kernel(**inputs) -> np.ndarray:
    nc = get_program()
    in_maps = _prep_core_inputs(**inputs)
    res = bass_utils.run_bass_kernel_spmd(nc, in_maps, core_ids=list(range(N_CORES)))
    out = np.zeros((B, HID, H, W, D), np.float32)
    for core in range(N_CORES):
        b = core // TP
        out[b] += res.results[core]["outT"].astype(np.float32).reshape(
            HID, H, W, D
        )
    out += np.asarray(inputs["x"], np.float32)
    return out
